# revision 12
# baseline (speedup 1.0000x reference)
"""Cross-attention Bass/Tile kernel for Trainium2, sharded over 8 NeuronCores.

Problem (fixed shapes): B=2, T=2048, C=1024, H=16 heads, D=64.
    q = x_q @ Wq + bq;  kv = x_kv @ Wkv + bkv;  k, v = split(kv)
    y = softmax(q k^T / sqrt(D)) v;  out = y @ Wo + bo

Sharding: 8 cores = 2 (batch) x 4 (head groups of 4 heads, 256 channels).
Each core computes its head-group's projections + attention + a partial
output projection (its 256 rows of Wo); the host sums the 4 partials per
batch and adds the exact bias terms (v-bias through Wo + output bias).

v2 design (vs baseline):
  - All activations/weights in bf16 (staged bf16 from the host); psum f32.
  - x^T produced by DMA-transpose (XBAR, 16x128 tiles) straight from DRAM
    into SBUF: no PE transposes, no DVE copy-outs.
  - att@V in the [q,d] orientation: lhsT = E[tk, q-chunk(128)] (M=128),
    rhs = V[tk, 64] (N=64) -> full PE efficiency (baseline had M=65).
  - Softmax denominators via N=1 matmuls (lhsT = same E chunk, rhs = ones)
    accumulated in a dedicated psum bank; normalization is a per-partition
    scalar DVE multiply fused into the psum->SBUF copy (no broadcast
    matmuls).
  - y^T for the output projection via SBUF->SBUF DMA-transpose.
  - Streamed schedule: K/V prep for token chunk tk feeds the tq0 passes'
    S/exp/att@V for tk directly; remaining passes weave Q-prep and the
    deferred output projection as filler, keeping ACT (exp, the ~133us
    floor) fed from ~t=10us on.

PSUM (8 banks): ps_s 2x[128,1024] (4) + ps_y 2x[128,512] (2) +
ps_w 1x[128,512] woven (1) + dn 1x[128,32] denominators (1).
"""

import numpy as np

B = 2
T = 2048
C = 1024
H = 16
D = 64
NCORES = 8
TPG = 4  # tensor-parallel group size (head groups)
HL = H // TPG  # heads per core = 4
CL = HL * D  # local channels = 256
P = 128

_CACHE = {}


def _build(debug=False):
    import concourse.tile as tile
    from concourse import bacc, mybir

    f32 = mybir.dt.float32
    bf16 = mybir.dt.bfloat16
    Exp = mybir.ActivationFunctionType.Exp

    nc = bacc.Bacc("TRN2", target_bir_lowering=False, debug=False)

    xq_d = nc.dram_tensor("xq", [T, C], bf16, kind="ExternalInput")
    xkv_d = nc.dram_tensor("xkv", [T, C], bf16, kind="ExternalInput")
    wq_d = nc.dram_tensor("wq", [C, CL], bf16, kind="ExternalInput")
    wk_d = nc.dram_tensor("wk", [C, CL], bf16, kind="ExternalInput")
    wv_d = nc.dram_tensor("wv", [C, CL], bf16, kind="ExternalInput")
    wo_d = nc.dram_tensor("wo", [CL, C], bf16, kind="ExternalInput")
    bq_d = nc.dram_tensor("bq", [CL], f32, kind="ExternalInput")
    bk_d = nc.dram_tensor("bk", [CL], f32, kind="ExternalInput")
    out_d = nc.dram_tensor("out", [T, C], bf16, kind="ExternalOutput")

    KC = C // P  # 8 contraction chunks for the projections
    NT = T // P  # 16 token chunks of 128
    NQ = 4  # tq blocks of 512
    QW = T // NQ  # 512
    DC = CL // P  # 2 chunks of local channels
    LAG = 5

    if debug:
        qt_d = nc.dram_tensor("qt", [P, DC, T], bf16, kind="ExternalOutput")
        kt_d = nc.dram_tensor("kt", [P, DC, T], bf16, kind="ExternalOutput")
        v_d = nc.dram_tensor("vd", [P, NT, HL, D], bf16, kind="ExternalOutput")
        yt_d = nc.dram_tensor("yt", [P, DC, T], bf16, kind="ExternalOutput")
        dn_d = nc.dram_tensor("dn", [P, 32], f32, kind="ExternalOutput")

    with tile.TileContext(nc) as tc:
        with (
            tc.tile_pool(name="const", bufs=1) as const,
            tc.tile_pool(name="persist", bufs=1) as persist,
            tc.tile_pool(name="xtp", bufs=2) as xtp,
            tc.tile_pool(name="ework", bufs=2 * NT + 2) as ework,
            tc.tile_pool(name="nrm", bufs=2) as nrm,
            tc.tile_pool(name="outst", bufs=3) as outst,
        ):
            ones1 = const.tile([P, 1], bf16)
            nc.vector.memset(ones1, 1.0)

            # ---- weights via SWDGE (Pool) so HWDGE queues stay free ----
            wq_sb = const.tile([P, KC, CL], bf16)
            wk_sb = const.tile([P, KC, CL], bf16)
            wv_sb = const.tile([P, KC, CL], bf16)
            wo_sb = const.tile([P, DC, C], bf16)
            for w_sb, w_d in ((wq_sb, wq_d), (wk_sb, wk_d), (wv_sb, wv_d)):
                src = w_d.rearrange("(o p) d -> p o d", p=P)
                for kc in range(KC):
                    nc.gpsimd.dma_start(w_sb[:, kc, :], src[:, kc, :])
            wo_src = wo_d.rearrange("(o p) n -> p o n", p=P)
            for dc in range(DC):
                nc.gpsimd.dma_start(wo_sb[:, dc, :], wo_src[:, dc, :])
            bq_sb = const.tile([P, DC], f32)
            bk_sb = const.tile([P, DC], f32)
            nc.gpsimd.dma_start(bq_sb, bq_d.rearrange("(o p) -> p o", p=P))
            nc.gpsimd.dma_start(bk_sb, bk_d.rearrange("(o p) -> p o", p=P))

            # ---- persistent activations (bf16) ----
            qt_sb = persist.tile([P, DC, T], bf16)  # Q^T [d, t]
            kt_sb = persist.tile([P, DC, T], bf16)  # K^T [d, t]
            v_sb = persist.tile([P, NT, HL, D], bf16)  # V [t, h, d]
            yt_sb = persist.tile([P, DC, T], bf16)  # y^T [d, t] normalized

            # rotating x^T granule buffers (one granule = 512 tokens)
            xq_t = {}
            xkv_t = {}

            # ---- input transposes (XBAR DMA), priority order ----
            def emit_xT(store, pool_tag, src_d, g):
                t0 = g * QW
                gt = xtp.tile([P, KC, QW], bf16, tag=pool_tag, name=f"x_{pool_tag}")
                store[g] = gt
                for c in range(KC):
                    nc.sync.dma_start(
                        gt[:, c, :],
                        src_d[t0 : t0 + QW, c * P : (c + 1) * P],
                        transpose=True,
                    )

            emit_xT(xq_t, "xq", xq_d, 0)
            emit_xT(xkv_t, "xkv", xkv_d, 0)
            for g in range(1, NQ):
                emit_xT(xkv_t, "xkv", xkv_d, g)
                emit_xT(xq_t, "xq", xq_d, g)

            # ---- PSUM pools ----
            ps_s = tc.alloc_tile_pool(name="ps_s", bufs=2, space="PSUM")
            ps_y = tc.alloc_tile_pool(name="ps_y", bufs=2, space="PSUM")
            ps_w = tc.alloc_tile_pool(name="ps_w", bufs=1, space="PSUM")
            ps_d = tc.alloc_tile_pool(name="ps_d", bufs=1, space="PSUM")
            dn_ps = ps_d.tile([P, 32], f32, tag="dn", name="dn_ps")

            # ---------- work units ----------
            def prep_unit(tk):
                """V + K projections for token chunk tk (one ps_w slot)."""
                g, lt = tk // 4, tk % 4
                xg = xkv_t[g]
                w = ps_w.tile([P, 512], f32, tag="w", name="w_prep")
                for c in range(KC):
                    nc.tensor.matmul(
                        w[:, 0:CL],
                        xg[:, c, lt * P : (lt + 1) * P],
                        wv_sb[:, c, :],
                        start=(c == 0),
                        stop=(c == KC - 1),
                    )
                for dc in range(DC):
                    for c in range(KC):
                        nc.tensor.matmul(
                            w[:, CL + dc * P : CL + (dc + 1) * P],
                            wk_sb[:, c, dc * P : (dc + 1) * P],
                            xg[:, c, lt * P : (lt + 1) * P],
                            start=(c == 0),
                            stop=(c == KC - 1),
                        )
                nc.vector.tensor_copy(
                    v_sb[:, tk, :, :],
                    w[:, 0:CL].rearrange("p (h d) -> p h d", h=HL),
                )
                for dc in range(DC):
                    nc.vector.tensor_scalar_add(
                        kt_sb[:, dc, tk * P : (tk + 1) * P],
                        w[:, CL + dc * P : CL + (dc + 1) * P],
                        bk_sb[:, dc : dc + 1],
                    )

            def qprep_units(tq):
                units = []
                for dc in range(DC):

                    def u(dc=dc):
                        w = ps_w.tile([P, 512], f32, tag="w", name="w_q")
                        for c in range(KC):
                            nc.tensor.matmul(
                                w,
                                wq_sb[:, c, dc * P : (dc + 1) * P],
                                xq_t[tq][:, c, :],
                                start=(c == 0),
                                stop=(c == KC - 1),
                            )
                        nc.vector.tensor_scalar_add(
                            qt_sb[:, dc, tq * QW : (tq + 1) * QW],
                            w,
                            bq_sb[:, dc : dc + 1],
                        )

                    units.append(u)
                return units

            def po_units(tq):
                units = []
                for tch in range(4):
                    for co in range(2):

                        def u(tch=tch, co=co):
                            w = ps_w.tile([P, 512], f32, tag="w", name="w_po")
                            t0 = tq * QW + tch * P
                            for dc in range(DC):
                                nc.tensor.matmul(
                                    w,
                                    yt_sb[:, dc, t0 : t0 + P],
                                    wo_sb[:, dc, co * QW : (co + 1) * QW],
                                    start=(dc == 0),
                                    stop=(dc == DC - 1),
                                )
                            o = outst.tile([P, QW], bf16, tag="o", name="o_st")
                            nc.vector.tensor_copy(o, w)
                            nc.sync.dma_start(
                                out_d[t0 : t0 + P, co * QW : (co + 1) * QW], o
                            )

                        units.append(u)
                return units

            # ---------- attention passes ----------
            passes = [(tq, hp) for tq in range(NQ) for hp in range(DC)]
            pass_index = {p: i for i, p in enumerate(passes)}
            e_tiles = {}
            unit_q = []

            def emit_S(p, tk):
                tq, hp = p
                sp = ps_s.tile([P, 2 * QW], f32, tag="s", name="sp")
                for hh in range(2):
                    nc.tensor.matmul(
                        sp[:, hh * QW : (hh + 1) * QW],
                        kt_sb[hh * 64 : (hh + 1) * 64, hp, tk * P : (tk + 1) * P],
                        qt_sb[hh * 64 : (hh + 1) * 64, hp, tq * QW : (tq + 1) * QW],
                        start=True,
                        stop=True,
                        tile_position=(hh * 64, 0),
                    )
                e2 = ework.tile([P, 2 * QW], bf16, tag="e", name="e2")
                nc.scalar.activation(e2, sp, Exp, scale=0.125)
                e_tiles[(p, tk)] = e2

            def wad_units(p):
                """att@V + denom for pass p: 8 contiguous accumulation
                blocks (psum group rule: no interleaving within a bank),
                then fused normalize + y^T DMA-transpose per qc."""
                tq, hp = p
                base = (pass_index[p] % 4) * 8
                state = {}

                def ublock(hh, qc):
                    def u():
                        if "y" not in state:
                            state["y"] = ps_y.tile([P, 512], f32, tag="y", name="y_ps")
                            state["rec"] = nrm.tile([P, 8], f32, tag="rc", name="rec")
                            state["yb"] = nrm.tile([P, 4, P], bf16, tag="yb", name="yb")
                        y = state["y"]
                        h = 2 * hp + hh
                        blk = hh * 4 + qc
                        off = hh * QW + qc * P
                        for tk in range(NT):
                            nc.tensor.matmul(
                                y[:, blk * 64 : (blk + 1) * 64],
                                e_tiles[(p, tk)][:, off : off + P],
                                v_sb[:, tk, h, :],
                                start=(tk == 0),
                                stop=(tk == NT - 1),
                            )
                        for tk in range(NT):
                            nc.tensor.matmul(
                                dn_ps[:, base + blk : base + blk + 1],
                                e_tiles[(p, tk)][:, off : off + P],
                                ones1,
                                start=(tk == 0),
                                stop=(tk == NT - 1),
                            )
                        # fused normalize for this block
                        with nc.allow_low_precision(reason="softmax reciprocal"):
                            nc.vector.reciprocal(
                                state["rec"][:, blk : blk + 1],
                                dn_ps[:, base + blk : base + blk + 1],
                            )
                        nc.vector.tensor_scalar_mul(
                            state["yb"][:, qc, hh * 64 : (hh + 1) * 64],
                            y[:, blk * 64 : (blk + 1) * 64],
                            state["rec"][:, blk : blk + 1],
                        )
                        if hh == 1:
                            # both heads of this qc normalized -> y^T block
                            nc.sync.dma_start(
                                yt_sb[:, hp, tq * QW + qc * P : tq * QW + (qc + 1) * P],
                                state["yb"][:, qc, :],
                                transpose=True,
                            )

                    return u

                units = [ublock(hh, qc) for qc in range(4) for hh in range(2)]

                def release():
                    for tk in range(NT):
                        e_tiles.pop((p, tk))

                units.append(release)
                return units

            # ---- stream phase: preps + pass (0,0) S/exp ----
            for u in qprep_units(0):
                u()
            prep_unit(0)
            for tk in range(1, NT + 1):
                if tk < NT:
                    prep_unit(tk)
                emit_S((0, 0), tk - 1)
                if tk == 8:
                    unit_q.extend(qprep_units(1))
                if unit_q:
                    unit_q.pop(0)()

            # ---- steady state: pass p streams S/exp, wad(p-1) + filler woven
            for p in passes[1:]:
                tq, hp = p
                prev = passes[pass_index[p] - 1]
                wq_units = wad_units(prev)
                for tk in range(NT):
                    emit_S(p, tk)
                    if wq_units:
                        wq_units.pop(0)()
                    elif unit_q:
                        unit_q.pop(0)()
                    if hp == 0 and tk == 10 and tq + 1 < NQ:
                        unit_q.extend(qprep_units(tq + 1))
                while wq_units:
                    wq_units.pop(0)()
                ptq, php = prev
                if php == 1:
                    unit_q.extend(po_units(ptq))

            # ---- tail: wad of the last pass, then its po ----
            for u in wad_units(passes[-1]):
                u()
            unit_q.extend(po_units(NQ - 1))
            while unit_q:
                unit_q.pop(0)()

            if debug:
                dnc = const.tile([P, 32], f32, name="dnc")
                nc.vector.tensor_copy(dnc, dn_ps)
                nc.sync.dma_start(qt_d[:, :, :], qt_sb)
                nc.sync.dma_start(kt_d[:, :, :], kt_sb)
                nc.sync.dma_start(v_d[:, :, :, :], v_sb)
                nc.sync.dma_start(yt_d[:, :, :], yt_sb)
                nc.sync.dma_start(dn_d[:, :], dnc)

            ps_d.release()
            ps_w.release()
            ps_y.release()
            ps_s.release()

    nc.compile()
    return nc


def _get_nc():
    if "nc" not in _CACHE:
        _CACHE["nc"] = _build()
    return _CACHE["nc"]


def _shard_inputs(x_q, x_kv, Wq, bq, Wkv, bkv, Wo):
    import ml_dtypes

    bf16 = ml_dtypes.bfloat16
    in_maps = []
    for core in range(NCORES):
        b = core // TPG
        g = core % TPG
        cols = slice(g * CL, (g + 1) * CL)
        in_maps.append(
            {
                "xq": np.ascontiguousarray(x_q[b]).astype(bf16),
                "xkv": np.ascontiguousarray(x_kv[b]).astype(bf16),
                "wq": np.ascontiguousarray(Wq[:, cols]).astype(bf16),
                "wk": np.ascontiguousarray(Wkv[:, :C][:, cols]).astype(bf16),
                "wv": np.ascontiguousarray(Wkv[:, C:][:, cols]).astype(bf16),
                "wo": np.ascontiguousarray(Wo[g * CL : (g + 1) * CL, :]).astype(bf16),
                "bq": np.ascontiguousarray(bq[cols]).astype(np.float32),
                "bk": np.ascontiguousarray(bkv[:C][cols]).astype(np.float32),
            }
        )
    return in_maps


def kernel(x_q, x_kv, Wq, bq, Wkv, bkv, Wo, bo):
    from concourse.bass_utils import run_bass_kernel_spmd

    x_q = np.asarray(x_q, dtype=np.float32)
    x_kv = np.asarray(x_kv, dtype=np.float32)
    Wq = np.asarray(Wq, dtype=np.float32)
    bq = np.asarray(bq, dtype=np.float32)
    Wkv = np.asarray(Wkv, dtype=np.float32)
    bkv = np.asarray(bkv, dtype=np.float32)
    Wo = np.asarray(Wo, dtype=np.float32)
    bo = np.asarray(bo, dtype=np.float32)

    nc = _get_nc()
    in_maps = _shard_inputs(x_q, x_kv, Wq, bq, Wkv, bkv, Wo)
    res = run_bass_kernel_spmd(nc, in_maps, core_ids=list(range(NCORES)))

    # host-side gather: sum tensor-parallel partials; add exact bias terms
    bias_full = bkv[C:] @ Wo + bo  # v-bias through Wo, plus output bias
    out = np.zeros((B, T, C), dtype=np.float32)
    for core in range(NCORES):
        out[core // TPG] += np.asarray(res.results[core]["out"]).astype(np.float32)
    out += bias_full[None, None, :]
    return out


# revision 13
# speedup vs baseline: 1.0943x; 1.0943x over previous
"""Cross-attention Bass/Tile kernel for Trainium2, sharded over 8 NeuronCores.

Problem (fixed shapes): B=2, T=2048, C=1024, H=16 heads, D=64.
    q = x_q @ Wq + bq;  kv = x_kv @ Wkv + bkv;  k, v = split(kv)
    y = softmax(q k^T / sqrt(D)) v;  out = y @ Wo + bo

Sharding: 8 cores = 2 (batch) x 4 (head groups of 4 heads, 256 channels).
Each core computes its head-group's projections + attention + a partial
output projection (its 256 rows of Wo); the host sums the 4 partials per
batch.  The v-bias and output bias are folded in exactly on the host:
    y = att@(V + 1*bv) = att@V + 1*bv   (att rows sum to 1)
    => out += bv @ Wo + bo              (added once per batch on the host)

v3 (over the f32r baseline):
  - bf16 operands everywhere (x and weights staged bf16 from host):
    halves DMA traffic; psum stays f32.
  - x^T via DMA-transpose (XBAR, 16x128 tiles) straight from DRAM to
    SBUF: eliminates all PE transposes (~49k cycles) and the DVE
    psum->SBUF copy-outs (~34us).
  - K projection at 512-token granularity (64 instead of 128 matmuls).
  - Output partials stored bf16 (halves store DMA).
  - Phase A (K/V prep) streams into the first attention pass as woven
    units instead of a serial prologue.

Attention per (tq 512-block, head-pair) pass, per tk chunk: S^T matmul
(2 heads row-packed via tile_position) -> exp on ACT (scale=1/8) ->
att@V matmuls lagging LAG units.  V carries a ones column so row 64 of
the y psum accumulates the softmax denominator; normalization is
reciprocal + K=1 broadcast matmul + DVE multiply (baseline-proven).
PE matmul count kept low (~850): the PE sequencer costs ~130ns per
instruction (SW decode), which is the binding constraint before engine
cycles for narrow matmuls.

PSUM (8 banks): 2 x [128,1024] "s" + 4 x [128,512] "y" slots shared by
y-accumulators and woven work units (baseline-proven rotation).
"""

import numpy as np

B = 2
T = 2048
C = 1024
H = 16
D = 64
NCORES = 8
TPG = 4  # tensor-parallel group size (head groups)
HL = H // TPG  # heads per core = 4
CL = HL * D  # local channels = 256
P = 128

_CACHE = {}


def _build(debug=False):
    import concourse.tile as tile
    from concourse import bacc, mybir

    f32 = mybir.dt.float32
    bf16 = mybir.dt.bfloat16
    Exp = mybir.ActivationFunctionType.Exp

    nc = bacc.Bacc("TRN2", target_bir_lowering=False, debug=False)

    xq_d = nc.dram_tensor("xq", [T, C], bf16, kind="ExternalInput")
    xkv_d = nc.dram_tensor("xkv", [T, C], bf16, kind="ExternalInput")
    wq_d = nc.dram_tensor("wq", [C, CL], bf16, kind="ExternalInput")
    wk_d = nc.dram_tensor("wk", [C, CL], bf16, kind="ExternalInput")
    wv_d = nc.dram_tensor("wv", [C, CL], bf16, kind="ExternalInput")
    wo_d = nc.dram_tensor("wo", [CL, C], bf16, kind="ExternalInput")
    bq_d = nc.dram_tensor("bq", [CL], f32, kind="ExternalInput")
    bk_d = nc.dram_tensor("bk", [CL], f32, kind="ExternalInput")
    out_d = nc.dram_tensor("out", [T, C], bf16, kind="ExternalOutput")

    KC = C // P  # 8 contraction chunks for the projections
    NT = T // P  # 16 token chunks of 128
    NQ = 4  # tq blocks of 512
    QW = T // NQ  # 512
    DC = CL // P  # 2 chunks of d_local
    LAG = 5

    with tile.TileContext(nc) as tc:
        with (
            tc.tile_pool(name="const", bufs=1) as const,
            tc.tile_pool(name="persist", bufs=1) as persist,
            tc.tile_pool(name="ework", bufs=7) as ework,
            tc.tile_pool(name="norm2", bufs=1) as norm2,
            tc.tile_pool(name="outst", bufs=3) as outst,
        ):
            ones4 = const.tile([P, HL, 1], bf16)
            nc.vector.memset(ones4, 1.0)
            onesb = const.tile([P, 64], bf16)
            nc.vector.memset(onesb, 1.0)

            # ---- weights via SWDGE so HWDGE is free for the x loads ----
            wq_sb = const.tile([P, KC, CL], bf16)
            wk_sb = const.tile([P, KC, CL], bf16)
            wv_sb = const.tile([P, KC, CL], bf16)
            wo_sb = const.tile([P, DC, C], bf16)
            for w_sb, w_d in ((wk_sb, wk_d), (wv_sb, wv_d), (wq_sb, wq_d)):
                src = w_d.rearrange("(o p) d -> p o d", p=P)
                for kc in range(KC):
                    nc.gpsimd.dma_start(w_sb[:, kc, :], src[:, kc, :])
            wo_src = wo_d.rearrange("(o p) n -> p o n", p=P)
            for dc in range(DC):
                nc.gpsimd.dma_start(wo_sb[:, dc, :], wo_src[:, dc, :])
            bq_sb = const.tile([P, DC], f32)
            bk_sb = const.tile([P, DC], f32)
            nc.gpsimd.dma_start(bq_sb, bq_d.rearrange("(o p) -> p o", p=P))
            nc.gpsimd.dma_start(bk_sb, bk_d.rearrange("(o p) -> p o", p=P))

            # ---- persistent activations ----
            xq_t = persist.tile([P, KC, T], bf16)  # xq^T  [c, t]
            xkv_t = persist.tile([P, KC, T], bf16)  # xkv^T [c, t]
            qt_sb = persist.tile([P, DC, T], bf16)  # Q^T  [d, t]
            kt_sb = persist.tile([P, DC, T], bf16)  # K^T  [d, t]
            v_sb = persist.tile([P, NT, HL, 66], bf16)  # V|1 [t, h, d+1]
            yt_sb = persist.tile([P, DC, T], bf16)  # y^T  [d, t] (normalized)

            # ---- input transposes (XBAR DMA), priority order ----
            def emit_xT(dst, src_d, g):
                t0 = g * QW
                for c in range(KC):
                    nc.sync.dma_start(
                        dst[:, c, t0 : t0 + QW],
                        src_d[t0 : t0 + QW, c * P : (c + 1) * P],
                        transpose=True,
                    )

            emit_xT(xq_t, xq_d, 0)
            emit_xT(xkv_t, xkv_d, 0)
            for g in range(1, NQ):
                emit_xT(xkv_t, xkv_d, g)
                emit_xT(xq_t, xq_d, g)

            # ---- kernel-wide PSUM: 2 x [128,1024] (s) + 4 x [128,512] (y)
            ps_s = tc.alloc_tile_pool(name="ps_s", bufs=2, space="PSUM")
            ps_y = tc.alloc_tile_pool(name="ps_y", bufs=4, space="PSUM")

            # ---------- emission helpers ----------
            def vproj_unit(tch):
                def u():
                    pv = ps_y.tile([P, QW], f32, tag="y", name="pv")
                    for c in range(KC):
                        nc.tensor.matmul(
                            pv[:, :CL],
                            xkv_t[:, c, tch * P : (tch + 1) * P],
                            wv_sb[:, c, :],
                            start=(c == 0),
                            stop=(c == KC - 1),
                        )
                    nc.vector.tensor_copy(
                        v_sb[:, tch, :, 0:64],
                        pv[:, :CL].rearrange("p (h d) -> p h d", h=HL),
                    )
                    nc.vector.tensor_copy(v_sb[:, tch, :, 64:65], ones4)

                return u

            def kproj_unit(g, dc):
                def u():
                    pp = ps_y.tile([P, QW], f32, tag="y", name="ppk")
                    for c in range(KC):
                        nc.tensor.matmul(
                            pp,
                            wk_sb[:, c, dc * P : (dc + 1) * P],
                            xkv_t[:, c, g * QW : (g + 1) * QW],
                            start=(c == 0),
                            stop=(c == KC - 1),
                        )
                    nc.vector.tensor_scalar_add(
                        kt_sb[:, dc, g * QW : (g + 1) * QW],
                        pp,
                        bk_sb[:, dc : dc + 1],
                    )

                return u

            def q_prep_units(tq):
                units = []
                for dc in range(DC):

                    def proj_u(dc=dc):
                        pp = ps_y.tile([P, QW], f32, tag="y", name="ppq")
                        for c in range(KC):
                            nc.tensor.matmul(
                                pp,
                                wq_sb[:, c, dc * P : (dc + 1) * P],
                                xq_t[:, c, tq * QW : (tq + 1) * QW],
                                start=(c == 0),
                                stop=(c == KC - 1),
                            )
                        nc.vector.tensor_scalar_add(
                            qt_sb[:, dc, tq * QW : (tq + 1) * QW],
                            pp,
                            bq_sb[:, dc : dc + 1],
                        )

                    units.append(proj_u)
                return units

            def po_units(tq):
                units = []
                for ts_ in range(4):
                    tch = tq * 4 + ts_
                    for co in range(2):

                        def u(tch=tch, co=co):
                            po = ps_y.tile([P, QW], f32, tag="y", name="po")
                            for dc in range(DC):
                                nc.tensor.matmul(
                                    po,
                                    yt_sb[:, dc, tch * P : (tch + 1) * P],
                                    wo_sb[:, dc, co * QW : (co + 1) * QW],
                                    start=(dc == 0),
                                    stop=(dc == DC - 1),
                                )
                            o_st = outst.tile([P, QW], bf16, tag="o")
                            nc.vector.tensor_copy(o_st, po)
                            nc.sync.dma_start(
                                out_d[
                                    tch * P : (tch + 1) * P, co * QW : (co + 1) * QW
                                ],
                                o_st,
                            )

                        units.append(u)
                return units

            # phase-A prep as a streamable queue: per granule g (512 tok):
            # 4 V-proj chunks + 2 K-proj halves
            prep_q = []
            for g in range(NQ):
                for ts_ in range(4):
                    prep_q.append(vproj_unit(g * 4 + ts_))
                for dc in range(DC):
                    prep_q.append(kproj_unit(g, dc))

            # ---- phase B: attention passes per (tq, head-pair) ----
            y_tiles = {}
            e_tiles = {}

            def emit_sexp(k, hc, tk):
                sp = ps_s.tile([P, 2 * QW], f32, tag="s", name="sp")
                for hh in range(2):
                    nc.tensor.matmul(
                        sp[:, hh * QW : (hh + 1) * QW],
                        kt_sb[hh * 64 : (hh + 1) * 64, hc, tk * P : (tk + 1) * P],
                        qt_sb[hh * 64 : (hh + 1) * 64, hc, k * QW : (k + 1) * QW],
                        start=True,
                        stop=True,
                        tile_position=(hh * 64, 0),
                    )
                e2 = ework.tile([P, 2 * QW], bf16, tag="e", name="e2")
                nc.scalar.activation(e2, sp, Exp, scale=0.125)
                e_tiles[(k, hc, tk)] = e2

            def emit_y(k, hc, tk):
                if (k, hc) not in y_tiles:
                    y_tiles[(k, hc)] = [
                        ps_y.tile([65, QW], f32, tag="y", name=f"y_ps{i}")
                        for i in range(2)
                    ]
                y_pair = y_tiles[(k, hc)]
                e2 = e_tiles.pop((k, hc, tk))
                for hh in range(2):
                    h = 2 * hc + hh
                    nc.tensor.matmul(
                        y_pair[hh],
                        v_sb[:, tk, h, :65],
                        e2[:, hh * QW : (hh + 1) * QW],
                        start=(tk == 0),
                        stop=(tk == NT - 1),
                    )

            def emit_norm(k, hc):
                y_pair = y_tiles.pop((k, hc))
                recr = norm2.tile([P, 2, QW], bf16, tag="recr")
                with nc.allow_low_precision(reason="softmax denom reciprocal"):
                    for hh in range(2):
                        nc.vector.reciprocal(
                            recr[64:65, hh, :], y_pair[hh][64:65, :]
                        )
                rbp = ps_s.tile([P, 2 * QW], f32, tag="s", name="rbp")
                for hh in range(2):
                    nc.tensor.matmul(
                        rbp[0:64, hh * QW : (hh + 1) * QW],
                        onesb[64:65, :],
                        recr[64:65, hh, :],
                        start=True,
                        stop=True,
                        tile_position=(64, 0),
                        skip_group_check=True,
                    )
                rbs = norm2.tile([P, 2 * QW], f32, tag="rbs")
                nc.vector.tensor_copy(rbs[0:64, :], rbp[0:64, :])
                for hh in range(2):
                    rb_h = rbs[0:64, hh * QW : (hh + 1) * QW]
                    if hh == 0:
                        nc.vector.tensor_mul(
                            out=yt_sb[0:64, hc, k * QW : (k + 1) * QW],
                            in0=y_pair[hh][0:64, :],
                            in1=rb_h,
                        )
                    else:
                        yst = norm2.tile([64, QW], bf16, tag="yst")
                        nc.vector.tensor_mul(
                            out=yst, in0=y_pair[hh][0:64, :], in1=rb_h
                        )
                        nc.sync.dma_start(
                            yt_sb[64:128, hc, k * QW : (k + 1) * QW], yst
                        )

            passes = [(k, hc) for k in range(NQ) for hc in range(DC)]
            unit_q = []
            yq = []
            # Q-prep for tq0 first (xq g0 is the first DMA), then granule-0
            # prep so S(0,0,0) has kt/v chunk 0
            for u in q_prep_units(0):
                u()
            for _ in range(6):
                prep_q.pop(0)()

            for pi, (k, hc) in enumerate(passes):
                if hc == 0:
                    while unit_q:
                        unit_q.pop(0)()
                    if k + 1 < NQ:
                        unit_q.extend(q_prep_units(k + 1))
                for tk in range(NT):
                    emit_sexp(k, hc, tk)
                    yq.append((k, hc, tk))
                    if len(yq) > LAG:
                        emit_y(*yq.pop(0))
                    if tk == 1 and pi >= 1:
                        pk, phc = passes[pi - 1]
                        while yq and yq[0][:2] == (pk, phc):
                            emit_y(*yq.pop(0))
                        emit_norm(pk, phc)
                        if hc == 0 and k >= 1:
                            unit_q.extend(po_units(k - 1))
                    # stream phase-A prep ahead of need during the first pass
                    if prep_q:
                        prep_q.pop(0)()
                        if tk % 2 == 0 and prep_q:
                            prep_q.pop(0)()
                    elif unit_q:
                        unit_q.pop(0)()
            while unit_q:
                unit_q.pop(0)()
            while yq:
                emit_y(*yq.pop(0))
            emit_norm(NQ - 1, DC - 1)
            for u in po_units(NQ - 1):
                u()

            ps_y.release()
            ps_s.release()

    nc.compile()
    return nc


def _get_nc():
    if "nc" not in _CACHE:
        _CACHE["nc"] = _build()
    return _CACHE["nc"]


def _shard_inputs(x_q, x_kv, Wq, bq, Wkv, bkv, Wo):
    import ml_dtypes

    bf16 = ml_dtypes.bfloat16
    in_maps = []
    for core in range(NCORES):
        b = core // TPG
        g = core % TPG
        cols = slice(g * CL, (g + 1) * CL)
        in_maps.append(
            {
                "xq": np.ascontiguousarray(x_q[b]).astype(bf16),
                "xkv": np.ascontiguousarray(x_kv[b]).astype(bf16),
                "wq": np.ascontiguousarray(Wq[:, cols]).astype(bf16),
                "wk": np.ascontiguousarray(Wkv[:, :C][:, cols]).astype(bf16),
                "wv": np.ascontiguousarray(Wkv[:, C:][:, cols]).astype(bf16),
                "wo": np.ascontiguousarray(Wo[g * CL : (g + 1) * CL, :]).astype(bf16),
                "bq": np.ascontiguousarray(bq[cols]).astype(np.float32),
                "bk": np.ascontiguousarray(bkv[:C][cols]).astype(np.float32),
            }
        )
    return in_maps


def kernel(x_q, x_kv, Wq, bq, Wkv, bkv, Wo, bo):
    from concourse.bass_utils import run_bass_kernel_spmd

    x_q = np.asarray(x_q, dtype=np.float32)
    x_kv = np.asarray(x_kv, dtype=np.float32)
    Wq = np.asarray(Wq, dtype=np.float32)
    bq = np.asarray(bq, dtype=np.float32)
    Wkv = np.asarray(Wkv, dtype=np.float32)
    bkv = np.asarray(bkv, dtype=np.float32)
    Wo = np.asarray(Wo, dtype=np.float32)
    bo = np.asarray(bo, dtype=np.float32)

    nc = _get_nc()
    in_maps = _shard_inputs(x_q, x_kv, Wq, bq, Wkv, bkv, Wo)
    res = run_bass_kernel_spmd(nc, in_maps, core_ids=list(range(NCORES)))

    # host-side gather: sum tensor-parallel partials; add exact bias terms
    bias_full = bkv[C:] @ Wo + bo  # v-bias through Wo, plus output bias
    out = np.zeros((B, T, C), dtype=np.float32)
    for core in range(NCORES):
        out[core // TPG] += np.asarray(res.results[core]["out"]).astype(np.float32)
    out += bias_full[None, None, :]
    return out


# revision 20
# speedup vs baseline: 1.7041x; 1.5573x over previous
"""Cross-attention Bass/Tile kernel for Trainium2, sharded over 8 NeuronCores.

Problem (fixed shapes): B=2, T=2048, C=1024, H=16 heads, D=64.
    q = x_q @ Wq + bq;  kv = x_kv @ Wkv + bkv;  k, v = split(kv)
    y = softmax(q k^T / sqrt(D)) v;  out = y @ Wo + bo

Sharding: 8 cores = 2 (batch) x 4 (head groups of 4 heads, 256 channels).
Each core computes its head-group's projections + attention + a partial
output projection (its 256 rows of Wo); the host sums the 4 partials per
batch.  The v-bias and output bias are folded in exactly on the host:
    y = att@(V + 1*bv) = att@V + 1*bv   (att rows sum to 1)
    => out += bv @ Wo + bo              (added once per batch on the host)

v3 (over the f32r baseline):
  - bf16 operands everywhere (x and weights staged bf16 from host):
    halves DMA traffic; psum stays f32.
  - x^T via DMA-transpose (XBAR, 16x128 tiles) straight from DRAM to
    SBUF: eliminates all PE transposes (~49k cycles) and the DVE
    psum->SBUF copy-outs (~34us).
  - K projection at 512-token granularity (64 instead of 128 matmuls).
  - Output partials stored bf16 (halves store DMA).
  - Phase A (K/V prep) streams into the first attention pass as woven
    units instead of a serial prologue.

Attention per (tq 512-block, head-pair) pass, per tk chunk: S^T matmul
(2 heads row-packed via tile_position) -> exp on ACT (scale=1/8) ->
att@V matmuls lagging LAG units.  V carries a ones column so row 64 of
the y psum accumulates the softmax denominator; normalization is
reciprocal + K=1 broadcast matmul + DVE multiply (baseline-proven).
PE matmul count kept low (~850): the PE sequencer costs ~130ns per
instruction (SW decode), which is the binding constraint before engine
cycles for narrow matmuls.

PSUM (8 banks): 2 x [128,1024] "s" + 4 x [128,512] "y" slots shared by
y-accumulators and woven work units (baseline-proven rotation).
"""

import numpy as np

B = 2
T = 2048
C = 1024
H = 16
D = 64
NCORES = 8
TPG = 4  # tensor-parallel group size (head groups)
HL = H // TPG  # heads per core = 4
CL = HL * D  # local channels = 256
P = 128

_CACHE = {}


def _build(debug=False):
    import concourse.tile as tile
    from concourse import bacc, mybir

    f32 = mybir.dt.float32
    bf16 = mybir.dt.bfloat16
    Exp = mybir.ActivationFunctionType.Exp

    nc = bacc.Bacc("TRN2", target_bir_lowering=False, debug=False)

    xq_d = nc.dram_tensor("xq", [T, C], bf16, kind="ExternalInput")
    xkv_d = nc.dram_tensor("xkv", [T, C], bf16, kind="ExternalInput")
    # all weights prepacked on host into one [128, 8192] bf16 blob:
    # [wq 8x256 | wk 8x256 | wv 8x256 | wo 2x1024] per partition row
    wb_d = nc.dram_tensor("wb", [P, 8192], bf16, kind="ExternalInput")
    bb_d = nc.dram_tensor("bb", [P, 4], f32, kind="ExternalInput")
    out_d = nc.dram_tensor("out", [T, C], bf16, kind="ExternalOutput")

    KC = C // P  # 8 contraction chunks for the projections
    NT = T // P  # 16 token chunks of 128
    NQ = 4  # tq blocks of 512
    QW = T // NQ  # 512
    DC = CL // P  # 2 chunks of d_local
    LAG = 5

    with tile.TileContext(nc) as tc:
        with (
            tc.tile_pool(name="const", bufs=1) as const,
            tc.tile_pool(name="persist", bufs=1) as persist,
            tc.tile_pool(name="ework", bufs=7) as ework,
            tc.tile_pool(name="norm2", bufs=1) as norm2,
            tc.tile_pool(name="outst", bufs=3) as outst,
        ):
            ones4 = const.tile([P, HL, 1], bf16)
            nc.vector.memset(ones4, 1.0)
            onesb = const.tile([P, 64], bf16)
            nc.vector.memset(onesb, 1.0)

            # ---- weights: ONE blob DMA + one bias DMA (DMA instructions
            # issue serially at ~2.7us each; count is precious) ----
            bb_sb = const.tile([P, 4], f32)
            nc.gpsimd.dma_start(bb_sb, bb_d[:, :])
            wb_sb = const.tile([P, 8192], bf16)
            nc.gpsimd.dma_start(wb_sb, wb_d[:, :])
            bq_sb = bb_sb[:, 0:2]
            bk_sb = bb_sb[:, 2:4]

            def wq_ap(kc, sl):
                return wb_sb[:, kc * CL + sl.start : kc * CL + sl.stop]

            def wk_ap(kc, sl):
                return wb_sb[:, 2048 + kc * CL + sl.start : 2048 + kc * CL + sl.stop]

            def wv_ap(kc):
                return wb_sb[:, 4096 + kc * CL : 4096 + (kc + 1) * CL]

            def wo_ap(dc, sl):
                return wb_sb[:, 6144 + dc * C + sl.start : 6144 + dc * C + sl.stop]

            # ---- persistent activations ----
            xq_t = persist.tile([P, KC, T], bf16)  # xq^T  [c, t]
            xkv_t = persist.tile([P, KC, T], bf16)  # xkv^T [c, t]
            qt_sb = persist.tile([P, DC, T], bf16)  # Q^T  [d, t]
            kt_sb = persist.tile([P, DC, T], bf16)  # K^T  [d, t]
            v_sb = persist.tile([P, NT, HL, 66], bf16)  # V|1 [t, h, d+1]
            yt_sb = persist.tile([P, DC, T], bf16)  # y^T  [d, t] (normalized)

            # ---- input transposes (XBAR DMA): ONE [512,1024] DMA per
            # granule covers all 8 c-chunks -> out[p, c, t] = x^T[c*128+p, t]
            def emit_xT(dst, src_d, g):
                t0 = g * QW
                nc.sync.dma_start(
                    dst[:, :, t0 : t0 + QW],
                    src_d[t0 : t0 + QW, :],
                    transpose=True,
                )

            emit_xT(xq_t, xq_d, 0)
            emit_xT(xkv_t, xkv_d, 0)
            for g in range(1, NQ):
                emit_xT(xkv_t, xkv_d, g)
            for g in range(1, NQ):
                emit_xT(xq_t, xq_d, g)

            # ---- kernel-wide PSUM: 2 x [128,1024] (s) + 4 x [128,512] (y)
            ps_s = tc.alloc_tile_pool(name="ps_s", bufs=2, space="PSUM")
            ps_y = tc.alloc_tile_pool(name="ps_y", bufs=4, space="PSUM")

            # ---------- emission helpers ----------
            def vproj_unit(tch):
                def u():
                    pv = ps_y.tile([P, QW], f32, tag="y", name="pv")
                    for c in range(KC):
                        nc.tensor.matmul(
                            pv[:, :CL],
                            xkv_t[:, c, tch * P : (tch + 1) * P],
                            wv_ap(c),
                            start=(c == 0),
                            stop=(c == KC - 1),
                        )
                    nc.vector.tensor_copy(
                        v_sb[:, tch, :, 0:64],
                        pv[:, :CL].rearrange("p (h d) -> p h d", h=HL),
                    )
                    nc.vector.tensor_copy(v_sb[:, tch, :, 64:65], ones4)

                return u

            def kproj_unit(g, dc):
                def u():
                    pp = ps_y.tile([P, QW], f32, tag="y", name="ppk")
                    for c in range(KC):
                        nc.tensor.matmul(
                            pp,
                            wk_ap(c, slice(dc * P, (dc + 1) * P)),
                            xkv_t[:, c, g * QW : (g + 1) * QW],
                            start=(c == 0),
                            stop=(c == KC - 1),
                        )
                    nc.vector.tensor_scalar_add(
                        kt_sb[:, dc, g * QW : (g + 1) * QW],
                        pp,
                        bk_sb[:, dc : dc + 1],
                    )

                return u

            def q_prep_units(tq):
                units = []
                for dc in range(DC):

                    def proj_u(dc=dc):
                        pp = ps_y.tile([P, QW], f32, tag="y", name="ppq")
                        for c in range(KC):
                            nc.tensor.matmul(
                                pp,
                                wq_ap(c, slice(dc * P, (dc + 1) * P)),
                                xq_t[:, c, tq * QW : (tq + 1) * QW],
                                start=(c == 0),
                                stop=(c == KC - 1),
                            )
                        nc.vector.tensor_scalar_add(
                            qt_sb[:, dc, tq * QW : (tq + 1) * QW],
                            pp,
                            bq_sb[:, dc : dc + 1],
                        )

                    units.append(proj_u)
                return units

            out_po = out_d.rearrange("(k f p) c -> k p f c", p=P, f=4)

            def po_units(tq):
                units = []
                state = {}
                for ts_ in range(4):
                    tch = tq * 4 + ts_
                    for co in range(2):

                        def u(tch=tch, ts_=ts_, co=co):
                            if "o" not in state:
                                state["o"] = outst.tile([P, 4, C], bf16, tag="o", name="o_st")
                            po = ps_y.tile([P, QW], f32, tag="y", name="po")
                            for dc in range(DC):
                                nc.tensor.matmul(
                                    po,
                                    yt_sb[:, dc, tch * P : (tch + 1) * P],
                                    wo_ap(dc, slice(co * QW, (co + 1) * QW)),
                                    start=(dc == 0),
                                    stop=(dc == DC - 1),
                                )
                            nc.vector.tensor_copy(
                                state["o"][:, ts_, co * QW : (co + 1) * QW], po
                            )
                            if ts_ == 3 and co == 1:
                                nc.sync.dma_start(out_po[tq], state["o"])

                        units.append(u)
                return units

            # phase-A prep as a streamable queue: per granule g (512 tok):
            # 4 V-proj chunks + 2 K-proj halves
            prep_q = []
            for g in range(NQ):
                for ts_ in range(4):
                    prep_q.append(vproj_unit(g * 4 + ts_))
                for dc in range(DC):
                    prep_q.append(kproj_unit(g, dc))

            # ---- phase B: attention passes per (tq, head-pair) ----
            y_tiles = {}
            e_tiles = {}

            def emit_sexp(k, hc, tk):
                sp = ps_s.tile([P, 2 * QW], f32, tag="s", name="sp")
                for hh in range(2):
                    nc.tensor.matmul(
                        sp[:, hh * QW : (hh + 1) * QW],
                        kt_sb[hh * 64 : (hh + 1) * 64, hc, tk * P : (tk + 1) * P],
                        qt_sb[hh * 64 : (hh + 1) * 64, hc, k * QW : (k + 1) * QW],
                        start=True,
                        stop=True,
                        tile_position=(hh * 64, 0),
                    )
                e2 = ework.tile([P, 2 * QW], bf16, tag="e", name="e2")
                nc.scalar.activation(e2, sp, Exp, scale=0.125)
                e_tiles[(k, hc, tk)] = e2

            def emit_y(k, hc, tk):
                if (k, hc) not in y_tiles:
                    y_tiles[(k, hc)] = [
                        ps_y.tile([65, QW], f32, tag="y", name=f"y_ps{i}")
                        for i in range(2)
                    ]
                y_pair = y_tiles[(k, hc)]
                e2 = e_tiles.pop((k, hc, tk))
                for hh in range(2):
                    h = 2 * hc + hh
                    nc.tensor.matmul(
                        y_pair[hh],
                        v_sb[:, tk, h, :65],
                        e2[:, hh * QW : (hh + 1) * QW],
                        start=(tk == 0),
                        stop=(tk == NT - 1),
                    )

            def emit_norm(k, hc):
                y_pair = y_tiles.pop((k, hc))
                recr = norm2.tile([P, 2, QW], bf16, tag="recr")
                with nc.allow_low_precision(reason="softmax denom reciprocal"):
                    for hh in range(2):
                        nc.vector.reciprocal(
                            recr[64:65, hh, :], y_pair[hh][64:65, :]
                        )
                rbp = ps_s.tile([P, 2 * QW], f32, tag="s", name="rbp")
                for hh in range(2):
                    nc.tensor.matmul(
                        rbp[0:64, hh * QW : (hh + 1) * QW],
                        onesb[64:65, :],
                        recr[64:65, hh, :],
                        start=True,
                        stop=True,
                        tile_position=(64, 0),
                        skip_group_check=True,
                    )
                rbs = norm2.tile([P, 2 * QW], f32, tag="rbs")
                nc.vector.tensor_copy(rbs[0:64, :], rbp[0:64, :])
                for hh in range(2):
                    rb_h = rbs[0:64, hh * QW : (hh + 1) * QW]
                    if hh == 0:
                        nc.vector.tensor_mul(
                            out=yt_sb[0:64, hc, k * QW : (k + 1) * QW],
                            in0=y_pair[hh][0:64, :],
                            in1=rb_h,
                        )
                    else:
                        yst = norm2.tile([64, QW], bf16, tag="yst")
                        nc.vector.tensor_mul(
                            out=yst, in0=y_pair[hh][0:64, :], in1=rb_h
                        )
                        nc.sync.dma_start(
                            yt_sb[64:128, hc, k * QW : (k + 1) * QW], yst
                        )

            passes = [(k, hc) for k in range(NQ) for hc in range(DC)]
            unit_q = []
            yq = []
            # Q-prep for tq0 first (xq g0 is the first DMA), then granule-0
            # prep so S(0,0,0) has kt/v chunk 0
            for u in q_prep_units(0):
                u()
            for _ in range(6):
                prep_q.pop(0)()

            for pi, (k, hc) in enumerate(passes):
                if hc == 0:
                    while unit_q:
                        unit_q.pop(0)()
                    if k + 1 < NQ:
                        unit_q.extend(q_prep_units(k + 1))
                for tk in range(NT):
                    emit_sexp(k, hc, tk)
                    yq.append((k, hc, tk))
                    if len(yq) > LAG:
                        emit_y(*yq.pop(0))
                    if tk == 1 and pi >= 1:
                        pk, phc = passes[pi - 1]
                        while yq and yq[0][:2] == (pk, phc):
                            emit_y(*yq.pop(0))
                        emit_norm(pk, phc)
                        if hc == 0 and k >= 1:
                            unit_q.extend(po_units(k - 1))
                    # stream phase-A prep ahead of need during the first pass
                    if prep_q:
                        prep_q.pop(0)()
                        if tk % 2 == 0 and prep_q:
                            prep_q.pop(0)()
                    elif unit_q:
                        unit_q.pop(0)()
            while unit_q:
                unit_q.pop(0)()
            while yq:
                emit_y(*yq.pop(0))
            emit_norm(NQ - 1, DC - 1)
            for u in po_units(NQ - 1):
                u()

            ps_y.release()
            ps_s.release()

    nc.compile()
    return nc


def _get_nc():
    if "nc" not in _CACHE:
        _CACHE["nc"] = _build()
    return _CACHE["nc"]


def _shard_inputs(x_q, x_kv, Wq, bq, Wkv, bkv, Wo):
    import ml_dtypes

    bf16 = ml_dtypes.bfloat16

    def pack_proj(W):  # [C, CL] -> [128, 8*256] in (kc, d) order
        return W.reshape(8, P, CL).transpose(1, 0, 2).reshape(P, 8 * CL)

    in_maps = []
    for core in range(NCORES):
        b = core // TPG
        g = core % TPG
        cols = slice(g * CL, (g + 1) * CL)
        wo_loc = Wo[g * CL : (g + 1) * CL, :]  # [256, 1024]
        wblob = np.concatenate(
            [
                pack_proj(Wq[:, cols]),
                pack_proj(Wkv[:, :C][:, cols]),
                pack_proj(Wkv[:, C:][:, cols]),
                wo_loc.reshape(2, P, C).transpose(1, 0, 2).reshape(P, 2 * C),
            ],
            axis=1,
        )
        bblob = np.concatenate(
            [
                bq[cols].reshape(2, P).T,
                bkv[:C][cols].reshape(2, P).T,
            ],
            axis=1,
        )
        in_maps.append(
            {
                "xq": np.ascontiguousarray(x_q[b]).astype(bf16),
                "xkv": np.ascontiguousarray(x_kv[b]).astype(bf16),
                "wb": np.ascontiguousarray(wblob).astype(bf16),
                "bb": np.ascontiguousarray(bblob).astype(np.float32),
            }
        )
    return in_maps


def kernel(x_q, x_kv, Wq, bq, Wkv, bkv, Wo, bo):
    from concourse.bass_utils import run_bass_kernel_spmd

    x_q = np.asarray(x_q, dtype=np.float32)
    x_kv = np.asarray(x_kv, dtype=np.float32)
    Wq = np.asarray(Wq, dtype=np.float32)
    bq = np.asarray(bq, dtype=np.float32)
    Wkv = np.asarray(Wkv, dtype=np.float32)
    bkv = np.asarray(bkv, dtype=np.float32)
    Wo = np.asarray(Wo, dtype=np.float32)
    bo = np.asarray(bo, dtype=np.float32)

    nc = _get_nc()
    in_maps = _shard_inputs(x_q, x_kv, Wq, bq, Wkv, bkv, Wo)
    res = run_bass_kernel_spmd(nc, in_maps, core_ids=list(range(NCORES)))

    # host-side gather: sum tensor-parallel partials; add exact bias terms
    bias_full = bkv[C:] @ Wo + bo  # v-bias through Wo, plus output bias
    out = np.zeros((B, T, C), dtype=np.float32)
    for core in range(NCORES):
        out[core // TPG] += np.asarray(res.results[core]["out"]).astype(np.float32)
    out += bias_full[None, None, :]
    return out


# revision 24
# speedup vs baseline: 1.7377x; 1.0197x over previous
"""Cross-attention Bass/Tile kernel for Trainium2, sharded over 8 NeuronCores.

Problem (fixed shapes): B=2, T=2048, C=1024, H=16 heads, D=64.
    q = x_q @ Wq + bq;  kv = x_kv @ Wkv + bkv;  k, v = split(kv)
    y = softmax(q k^T / sqrt(D)) v;  out = y @ Wo + bo

Sharding: 8 cores = 2 (batch) x 4 (head groups of 4 heads, 256 channels).
Each core computes its head-group's projections + attention + a partial
output projection (its 256 rows of Wo); the host sums the 4 partials per
batch.  The v-bias and output bias are folded in exactly on the host:
    y = att@(V + 1*bv) = att@V + 1*bv   (att rows sum to 1)
    => out += bv @ Wo + bo              (added once per batch on the host)

v3 (over the f32r baseline):
  - bf16 operands everywhere (x and weights staged bf16 from host):
    halves DMA traffic; psum stays f32.
  - x^T via DMA-transpose (XBAR, 16x128 tiles) straight from DRAM to
    SBUF: eliminates all PE transposes (~49k cycles) and the DVE
    psum->SBUF copy-outs (~34us).
  - K projection at 512-token granularity (64 instead of 128 matmuls).
  - Output partials stored bf16 (halves store DMA).
  - Phase A (K/V prep) streams into the first attention pass as woven
    units instead of a serial prologue.

Attention per (tq 512-block, head-pair) pass, per tk chunk: S^T matmul
(2 heads row-packed via tile_position) -> exp on ACT (scale=1/8) ->
att@V matmuls lagging LAG units.  V carries a ones column so row 64 of
the y psum accumulates the softmax denominator; normalization is
reciprocal + K=1 broadcast matmul + DVE multiply (baseline-proven).
PE matmul count kept low (~850): the PE sequencer costs ~130ns per
instruction (SW decode), which is the binding constraint before engine
cycles for narrow matmuls.

PSUM (8 banks): 2 x [128,1024] "s" + 4 x [128,512] "y" slots shared by
y-accumulators and woven work units (baseline-proven rotation).
"""

import numpy as np

B = 2
T = 2048
C = 1024
H = 16
D = 64
NCORES = 8
TPG = 4  # tensor-parallel group size (head groups)
HL = H // TPG  # heads per core = 4
CL = HL * D  # local channels = 256
P = 128

_CACHE = {}


def _build(debug=False):
    import concourse.tile as tile
    from concourse import bacc, mybir

    f32 = mybir.dt.float32
    bf16 = mybir.dt.bfloat16
    Exp = mybir.ActivationFunctionType.Exp

    nc = bacc.Bacc("TRN2", target_bir_lowering=False, debug=False)

    xq_d = nc.dram_tensor("xq", [T, C], bf16, kind="ExternalInput")
    xkv_d = nc.dram_tensor("xkv", [T, C], bf16, kind="ExternalInput")
    # weights prepacked on host into two bf16 blobs: wb1=[wq 8x256],
    # wb2=[wk 8x256 | wv 8x256 | wo 2x1024] per partition row (wb1 first
    # so Q-prep's DMA chain is short)
    wb1_d = nc.dram_tensor("wb1", [P, 2048], bf16, kind="ExternalInput")
    wb2_d = nc.dram_tensor("wb2", [P, 6144], bf16, kind="ExternalInput")
    bb_d = nc.dram_tensor("bb", [P, 4], f32, kind="ExternalInput")
    out_d = nc.dram_tensor("out", [T, C], bf16, kind="ExternalOutput")

    KC = C // P  # 8 contraction chunks for the projections
    NT = T // P  # 16 token chunks of 128
    NQ = 4  # tq blocks of 512
    QW = T // NQ  # 512
    DC = CL // P  # 2 chunks of d_local
    LAG = 5

    with tile.TileContext(nc) as tc:
        with (
            tc.tile_pool(name="const", bufs=1) as const,
            tc.tile_pool(name="persist", bufs=1) as persist,
            tc.tile_pool(name="ework", bufs=7) as ework,
            tc.tile_pool(name="norm2", bufs=1) as norm2,
            tc.tile_pool(name="outst", bufs=3) as outst,
        ):
            ones4 = const.tile([P, HL, 1], bf16)
            nc.vector.memset(ones4, 1.0)
            onesb = const.tile([P, 64], bf16)
            nc.vector.memset(onesb, 1.0)

            # ---- weights: ONE blob DMA + one bias DMA (DMA instructions
            # issue serially at ~2.7us each; count is precious) ----
            bb_sb = const.tile([P, 4], f32)
            nc.gpsimd.dma_start(bb_sb, bb_d[:, :])
            wb1_sb = const.tile([P, 2048], bf16)
            nc.gpsimd.dma_start(wb1_sb, wb1_d[:, :])
            wb2_sb = const.tile([P, 6144], bf16)
            bq_sb = bb_sb[:, 0:2]
            bk_sb = bb_sb[:, 2:4]

            def wq_ap(kc, sl):
                return wb1_sb[:, kc * CL + sl.start : kc * CL + sl.stop]

            def wk_ap(kc, sl):
                return wb2_sb[:, kc * CL + sl.start : kc * CL + sl.stop]

            def wv_ap(kc):
                return wb2_sb[:, 2048 + kc * CL : 2048 + (kc + 1) * CL]

            def wo_ap(dc, sl):
                return wb2_sb[:, 4096 + dc * C + sl.start : 4096 + dc * C + sl.stop]

            # ---- persistent activations ----
            xq_t = persist.tile([P, KC, T], bf16)  # xq^T  [c, t]
            xkv_t = persist.tile([P, KC, T], bf16)  # xkv^T [c, t]
            qt_sb = persist.tile([P, DC, T], bf16)  # Q^T  [d, t]
            kt_sb = persist.tile([P, DC, T], bf16)  # K^T  [d, t]
            v_sb = persist.tile([P, NT, HL, 66], bf16)  # V|1 [t, h, d+1]
            yt_sb = persist.tile([P, DC, T], bf16)  # y^T  [d, t] (normalized)

            # ---- input transposes (XBAR DMA): ONE [512,1024] DMA per
            # granule covers all 8 c-chunks -> out[p, c, t] = x^T[c*128+p, t]
            def emit_xT(dst, src_d, g):
                t0 = g * QW
                nc.sync.dma_start(
                    dst[:, :, t0 : t0 + QW],
                    src_d[t0 : t0 + QW, :],
                    transpose=True,
                )

            emit_xT(xq_t, xq_d, 0)
            nc.gpsimd.dma_start(wb2_sb, wb2_d[:, :])
            emit_xT(xkv_t, xkv_d, 0)
            for g in range(1, NQ):
                emit_xT(xkv_t, xkv_d, g)
            for g in range(1, NQ):
                emit_xT(xq_t, xq_d, g)

            # ---- kernel-wide PSUM: 2 x [128,1024] (s) + 4 x [128,512] (y)
            ps_s = tc.alloc_tile_pool(name="ps_s", bufs=2, space="PSUM")
            ps_y = tc.alloc_tile_pool(name="ps_y", bufs=4, space="PSUM")

            # ---------- emission helpers ----------
            def vproj_unit(tch):
                def u():
                    pv = ps_y.tile([P, QW], f32, tag="y", name="pv")
                    for c in range(KC):
                        nc.tensor.matmul(
                            pv[:, :CL],
                            xkv_t[:, c, tch * P : (tch + 1) * P],
                            wv_ap(c),
                            start=(c == 0),
                            stop=(c == KC - 1),
                        )
                    nc.vector.tensor_copy(
                        v_sb[:, tch, :, 0:64],
                        pv[:, :CL].rearrange("p (h d) -> p h d", h=HL),
                    )
                    nc.vector.tensor_copy(v_sb[:, tch, :, 64:65], ones4)

                return u

            def kproj_unit(g, dc):
                def u():
                    pp = ps_y.tile([P, QW], f32, tag="y", name="ppk")
                    for c in range(KC):
                        nc.tensor.matmul(
                            pp,
                            wk_ap(c, slice(dc * P, (dc + 1) * P)),
                            xkv_t[:, c, g * QW : (g + 1) * QW],
                            start=(c == 0),
                            stop=(c == KC - 1),
                        )
                    nc.vector.tensor_scalar_add(
                        kt_sb[:, dc, g * QW : (g + 1) * QW],
                        pp,
                        bk_sb[:, dc : dc + 1],
                    )

                return u

            def q_prep_units(tq):
                units = []
                for dc in range(DC):

                    def proj_u(dc=dc):
                        pp = ps_y.tile([P, QW], f32, tag="y", name="ppq")
                        for c in range(KC):
                            nc.tensor.matmul(
                                pp,
                                wq_ap(c, slice(dc * P, (dc + 1) * P)),
                                xq_t[:, c, tq * QW : (tq + 1) * QW],
                                start=(c == 0),
                                stop=(c == KC - 1),
                            )
                        nc.vector.tensor_scalar_add(
                            qt_sb[:, dc, tq * QW : (tq + 1) * QW],
                            pp,
                            bq_sb[:, dc : dc + 1],
                        )

                    units.append(proj_u)
                return units

            out_po = out_d.rearrange("(k f p) c -> k p f c", p=P, f=4)

            def po_units(tq):
                units = []
                state = {}
                for ts_ in range(4):
                    tch = tq * 4 + ts_
                    for co in range(2):

                        def u(tch=tch, ts_=ts_, co=co):
                            if "o" not in state:
                                state["o"] = outst.tile([P, 4, C], bf16, tag="o", name="o_st")
                            po = ps_y.tile([P, QW], f32, tag="y", name="po")
                            for dc in range(DC):
                                nc.tensor.matmul(
                                    po,
                                    yt_sb[:, dc, tch * P : (tch + 1) * P],
                                    wo_ap(dc, slice(co * QW, (co + 1) * QW)),
                                    start=(dc == 0),
                                    stop=(dc == DC - 1),
                                )
                            nc.vector.tensor_copy(
                                state["o"][:, ts_, co * QW : (co + 1) * QW], po
                            )
                            if ts_ == 3 and co == 1:
                                nc.sync.dma_start(out_po[tq], state["o"])

                        units.append(u)
                return units

            # phase-A prep as a streamable queue: per granule g (512 tok):
            # 4 V-proj chunks + 2 K-proj halves
            prep_q = []
            for g in range(NQ):
                for ts_ in range(4):
                    prep_q.append(vproj_unit(g * 4 + ts_))
                for dc in range(DC):
                    prep_q.append(kproj_unit(g, dc))

            # ---- phase B: attention passes per (tq, head-pair) ----
            y_tiles = {}
            e_tiles = {}

            def emit_sexp(k, hc, tk):
                sp = ps_s.tile([P, 2 * QW], f32, tag="s", name="sp")
                for hh in range(2):
                    nc.tensor.matmul(
                        sp[:, hh * QW : (hh + 1) * QW],
                        kt_sb[hh * 64 : (hh + 1) * 64, hc, tk * P : (tk + 1) * P],
                        qt_sb[hh * 64 : (hh + 1) * 64, hc, k * QW : (k + 1) * QW],
                        start=True,
                        stop=True,
                        tile_position=(hh * 64, 0),
                    )
                e2 = ework.tile([P, 2 * QW], bf16, tag="e", name="e2")
                nc.scalar.activation(e2, sp, Exp, scale=0.125)
                e_tiles[(k, hc, tk)] = e2

            def emit_y(k, hc, tk):
                if (k, hc) not in y_tiles:
                    y_tiles[(k, hc)] = [
                        ps_y.tile([65, QW], f32, tag="y", name=f"y_ps{i}")
                        for i in range(2)
                    ]
                y_pair = y_tiles[(k, hc)]
                e2 = e_tiles.pop((k, hc, tk))
                for hh in range(2):
                    h = 2 * hc + hh
                    nc.tensor.matmul(
                        y_pair[hh],
                        v_sb[:, tk, h, :65],
                        e2[:, hh * QW : (hh + 1) * QW],
                        start=(tk == 0),
                        stop=(tk == NT - 1),
                    )

            def emit_norm(k, hc):
                y_pair = y_tiles.pop((k, hc))
                recr = norm2.tile([P, 2, QW], bf16, tag="recr")
                with nc.allow_low_precision(reason="softmax denom reciprocal"):
                    for hh in range(2):
                        nc.vector.reciprocal(
                            recr[64:65, hh, :], y_pair[hh][64:65, :]
                        )
                rbp = ps_s.tile([P, 2 * QW], f32, tag="s", name="rbp")
                for hh in range(2):
                    nc.tensor.matmul(
                        rbp[0:64, hh * QW : (hh + 1) * QW],
                        onesb[64:65, :],
                        recr[64:65, hh, :],
                        start=True,
                        stop=True,
                        tile_position=(64, 0),
                        skip_group_check=True,
                    )
                rbs = norm2.tile([P, 2 * QW], f32, tag="rbs")
                nc.vector.tensor_copy(rbs[0:64, :], rbp[0:64, :])
                for hh in range(2):
                    rb_h = rbs[0:64, hh * QW : (hh + 1) * QW]
                    if hh == 0:
                        nc.vector.tensor_mul(
                            out=yt_sb[0:64, hc, k * QW : (k + 1) * QW],
                            in0=y_pair[hh][0:64, :],
                            in1=rb_h,
                        )
                    else:
                        yst = norm2.tile([64, QW], bf16, tag="yst")
                        nc.vector.tensor_mul(
                            out=yst, in0=y_pair[hh][0:64, :], in1=rb_h
                        )
                        nc.sync.dma_start(
                            yt_sb[64:128, hc, k * QW : (k + 1) * QW], yst
                        )

            passes = [(k, hc) for k in range(NQ) for hc in range(DC)]
            unit_q = []
            yq = []
            # Q-prep for tq0 first (xq g0 is the first DMA), then granule-0
            # prep so S(0,0,0) has kt/v chunk 0
            for u in q_prep_units(0):
                u()
            for _ in range(6):
                prep_q.pop(0)()

            for pi, (k, hc) in enumerate(passes):
                if hc == 0:
                    while unit_q:
                        unit_q.pop(0)()
                    if k + 1 < NQ:
                        unit_q.extend(q_prep_units(k + 1))
                for tk in range(NT):
                    emit_sexp(k, hc, tk)
                    yq.append((k, hc, tk))
                    if len(yq) > LAG:
                        emit_y(*yq.pop(0))
                    if tk == 1 and pi >= 1:
                        pk, phc = passes[pi - 1]
                        while yq and yq[0][:2] == (pk, phc):
                            emit_y(*yq.pop(0))
                        emit_norm(pk, phc)
                        if hc == 0 and k >= 1:
                            unit_q.extend(po_units(k - 1))
                    # stream phase-A prep ahead of need during the first pass
                    if prep_q:
                        prep_q.pop(0)()
                        if tk % 2 == 0 and prep_q:
                            prep_q.pop(0)()
                    elif unit_q:
                        unit_q.pop(0)()
            while unit_q:
                unit_q.pop(0)()
            while yq:
                emit_y(*yq.pop(0))
            emit_norm(NQ - 1, DC - 1)
            for u in po_units(NQ - 1):
                u()

            ps_y.release()
            ps_s.release()

    nc.compile()
    return nc


def _get_nc():
    if "nc" not in _CACHE:
        _CACHE["nc"] = _build()
    return _CACHE["nc"]


def _shard_inputs(x_q, x_kv, Wq, bq, Wkv, bkv, Wo):
    import ml_dtypes

    bf16 = ml_dtypes.bfloat16

    def pack_proj(W):  # [C, CL] -> [128, 8*256] in (kc, d) order
        return W.reshape(8, P, CL).transpose(1, 0, 2).reshape(P, 8 * CL)

    in_maps = []
    for core in range(NCORES):
        b = core // TPG
        g = core % TPG
        cols = slice(g * CL, (g + 1) * CL)
        wo_loc = Wo[g * CL : (g + 1) * CL, :]  # [256, 1024]
        wblob1 = pack_proj(Wq[:, cols])
        wblob2 = np.concatenate(
            [
                pack_proj(Wkv[:, :C][:, cols]),
                pack_proj(Wkv[:, C:][:, cols]),
                wo_loc.reshape(2, P, C).transpose(1, 0, 2).reshape(P, 2 * C),
            ],
            axis=1,
        )
        bblob = np.concatenate(
            [
                bq[cols].reshape(2, P).T,
                bkv[:C][cols].reshape(2, P).T,
            ],
            axis=1,
        )
        in_maps.append(
            {
                "xq": np.ascontiguousarray(x_q[b]).astype(bf16),
                "xkv": np.ascontiguousarray(x_kv[b]).astype(bf16),
                "wb1": np.ascontiguousarray(wblob1).astype(bf16),
                "wb2": np.ascontiguousarray(wblob2).astype(bf16),
                "bb": np.ascontiguousarray(bblob).astype(np.float32),
            }
        )
    return in_maps


def kernel(x_q, x_kv, Wq, bq, Wkv, bkv, Wo, bo):
    from concourse.bass_utils import run_bass_kernel_spmd

    x_q = np.asarray(x_q, dtype=np.float32)
    x_kv = np.asarray(x_kv, dtype=np.float32)
    Wq = np.asarray(Wq, dtype=np.float32)
    bq = np.asarray(bq, dtype=np.float32)
    Wkv = np.asarray(Wkv, dtype=np.float32)
    bkv = np.asarray(bkv, dtype=np.float32)
    Wo = np.asarray(Wo, dtype=np.float32)
    bo = np.asarray(bo, dtype=np.float32)

    nc = _get_nc()
    in_maps = _shard_inputs(x_q, x_kv, Wq, bq, Wkv, bkv, Wo)
    res = run_bass_kernel_spmd(nc, in_maps, core_ids=list(range(NCORES)))

    # host-side gather: sum tensor-parallel partials; add exact bias terms
    bias_full = bkv[C:] @ Wo + bo  # v-bias through Wo, plus output bias
    out = np.zeros((B, T, C), dtype=np.float32)
    for core in range(NCORES):
        out[core // TPG] += np.asarray(res.results[core]["out"]).astype(np.float32)
    out += bias_full[None, None, :]
    return out


# revision 25
# speedup vs baseline: 1.7437x; 1.0035x over previous
"""Cross-attention Bass/Tile kernel for Trainium2, sharded over 8 NeuronCores.

Problem (fixed shapes): B=2, T=2048, C=1024, H=16 heads, D=64.
    q = x_q @ Wq + bq;  kv = x_kv @ Wkv + bkv;  k, v = split(kv)
    y = softmax(q k^T / sqrt(D)) v;  out = y @ Wo + bo

Sharding: 8 cores = 2 (batch) x 4 (head groups of 4 heads, 256 channels).
Each core computes its head-group's projections + attention + a partial
output projection (its 256 rows of Wo); the host sums the 4 partials per
batch.  The v-bias and output bias are folded in exactly on the host:
    y = att@(V + 1*bv) = att@V + 1*bv   (att rows sum to 1)
    => out += bv @ Wo + bo              (added once per batch on the host)

v3 (over the f32r baseline):
  - bf16 operands everywhere (x and weights staged bf16 from host):
    halves DMA traffic; psum stays f32.
  - x^T via DMA-transpose (XBAR, 16x128 tiles) straight from DRAM to
    SBUF: eliminates all PE transposes (~49k cycles) and the DVE
    psum->SBUF copy-outs (~34us).
  - K projection at 512-token granularity (64 instead of 128 matmuls).
  - Output partials stored bf16 (halves store DMA).
  - Phase A (K/V prep) streams into the first attention pass as woven
    units instead of a serial prologue.

Attention per (tq 512-block, head-pair) pass, per tk chunk: S^T matmul
(2 heads row-packed via tile_position) -> exp on ACT (scale=1/8) ->
att@V matmuls lagging LAG units.  V carries a ones column so row 64 of
the y psum accumulates the softmax denominator; normalization is
reciprocal + K=1 broadcast matmul + DVE multiply (baseline-proven).
PE matmul count kept low (~850): the PE sequencer costs ~130ns per
instruction (SW decode), which is the binding constraint before engine
cycles for narrow matmuls.

PSUM (8 banks): 2 x [128,1024] "s" + 4 x [128,512] "y" slots shared by
y-accumulators and woven work units (baseline-proven rotation).
"""

import numpy as np

B = 2
T = 2048
C = 1024
H = 16
D = 64
NCORES = 8
TPG = 4  # tensor-parallel group size (head groups)
HL = H // TPG  # heads per core = 4
CL = HL * D  # local channels = 256
P = 128

_CACHE = {}


def _build(debug=False):
    import concourse.tile as tile
    from concourse import bacc, mybir

    f32 = mybir.dt.float32
    bf16 = mybir.dt.bfloat16
    Exp = mybir.ActivationFunctionType.Exp

    nc = bacc.Bacc("TRN2", target_bir_lowering=False, debug=False)

    xq_d = nc.dram_tensor("xq", [T, C], bf16, kind="ExternalInput")
    xkv_d = nc.dram_tensor("xkv", [T, C], bf16, kind="ExternalInput")
    # weights prepacked on host into two bf16 blobs: wb1=[wq 8x256],
    # wb2=[wk 8x256 | wv 8x256 | wo 2x1024] per partition row (wb1 first
    # so Q-prep's DMA chain is short)
    wb1_d = nc.dram_tensor("wb1", [P, 2048], bf16, kind="ExternalInput")
    wb2_d = nc.dram_tensor("wb2", [P, 6144], bf16, kind="ExternalInput")
    bb_d = nc.dram_tensor("bb", [P, 4], f32, kind="ExternalInput")
    out_d = nc.dram_tensor("out", [T, C], bf16, kind="ExternalOutput")

    KC = C // P  # 8 contraction chunks for the projections
    NT = T // P  # 16 token chunks of 128
    NQ = 4  # tq blocks of 512
    QW = T // NQ  # 512
    DC = CL // P  # 2 chunks of d_local
    LAG = 5

    with tile.TileContext(nc) as tc:
        with (
            tc.tile_pool(name="const", bufs=1) as const,
            tc.tile_pool(name="persist", bufs=1) as persist,
            tc.tile_pool(name="ework", bufs=7) as ework,
            tc.tile_pool(name="norm2", bufs=1) as norm2,
            tc.tile_pool(name="outst", bufs=3) as outst,
        ):
            ones4 = const.tile([P, HL, 1], bf16)
            nc.vector.memset(ones4, 1.0)
            onesb = const.tile([P, 64], bf16)
            nc.vector.memset(onesb, 1.0)

            # ---- weights: ONE blob DMA + one bias DMA (DMA instructions
            # issue serially at ~2.7us each; count is precious) ----
            bb_sb = const.tile([P, 4], f32)
            wb1_sb = const.tile([P, 2048], bf16)
            nc.gpsimd.dma_start(wb1_sb, wb1_d[:, :])
            wb2_sb = const.tile([P, 6144], bf16)
            bq_sb = bb_sb[:, 0:2]
            bk_sb = bb_sb[:, 2:4]

            def wq_ap(kc, sl):
                return wb1_sb[:, kc * CL + sl.start : kc * CL + sl.stop]

            def wk_ap(kc, sl):
                return wb2_sb[:, kc * CL + sl.start : kc * CL + sl.stop]

            def wv_ap(kc):
                return wb2_sb[:, 2048 + kc * CL : 2048 + (kc + 1) * CL]

            def wo_ap(dc, sl):
                return wb2_sb[:, 4096 + dc * C + sl.start : 4096 + dc * C + sl.stop]

            # ---- persistent activations ----
            xq_t = persist.tile([P, KC, T], bf16)  # xq^T  [c, t]
            xkv_t = persist.tile([P, KC, T], bf16)  # xkv^T [c, t]
            qt_sb = persist.tile([P, DC, T], bf16)  # Q^T  [d, t]
            kt_sb = persist.tile([P, DC, T], bf16)  # K^T  [d, t]
            v_sb = persist.tile([P, NT, HL, 66], bf16)  # V|1 [t, h, d+1]
            yt_sb = persist.tile([P, DC, T], bf16)  # y^T  [d, t] (normalized)

            # ---- input transposes (XBAR DMA): ONE [512,1024] DMA per
            # granule covers all 8 c-chunks -> out[p, c, t] = x^T[c*128+p, t]
            def emit_xT(dst, src_d, g):
                t0 = g * QW
                nc.sync.dma_start(
                    dst[:, :, t0 : t0 + QW],
                    src_d[t0 : t0 + QW, :],
                    transpose=True,
                )

            emit_xT(xq_t, xq_d, 0)
            nc.gpsimd.dma_start(bb_sb, bb_d[:, :])
            nc.gpsimd.dma_start(wb2_sb, wb2_d[:, :])
            emit_xT(xkv_t, xkv_d, 0)
            for g in range(1, NQ):
                emit_xT(xkv_t, xkv_d, g)
            for g in range(1, NQ):
                emit_xT(xq_t, xq_d, g)

            # ---- kernel-wide PSUM: 2 x [128,1024] (s) + 4 x [128,512] (y)
            ps_s = tc.alloc_tile_pool(name="ps_s", bufs=2, space="PSUM")
            ps_y = tc.alloc_tile_pool(name="ps_y", bufs=4, space="PSUM")

            # ---------- emission helpers ----------
            def vproj_unit(tch):
                def u():
                    pv = ps_y.tile([P, QW], f32, tag="y", name="pv")
                    for c in range(KC):
                        nc.tensor.matmul(
                            pv[:, :CL],
                            xkv_t[:, c, tch * P : (tch + 1) * P],
                            wv_ap(c),
                            start=(c == 0),
                            stop=(c == KC - 1),
                        )
                    nc.vector.tensor_copy(
                        v_sb[:, tch, :, 0:64],
                        pv[:, :CL].rearrange("p (h d) -> p h d", h=HL),
                    )
                    nc.vector.tensor_copy(v_sb[:, tch, :, 64:65], ones4)

                return u

            def kproj_unit(g, dc):
                def u():
                    pp = ps_y.tile([P, QW], f32, tag="y", name="ppk")
                    for c in range(KC):
                        nc.tensor.matmul(
                            pp,
                            wk_ap(c, slice(dc * P, (dc + 1) * P)),
                            xkv_t[:, c, g * QW : (g + 1) * QW],
                            start=(c == 0),
                            stop=(c == KC - 1),
                        )
                    nc.vector.tensor_scalar_add(
                        kt_sb[:, dc, g * QW : (g + 1) * QW],
                        pp,
                        bk_sb[:, dc : dc + 1],
                    )

                return u

            def q_prep_units(tq):
                units = []
                for dc in range(DC):

                    def proj_u(dc=dc):
                        pp = ps_y.tile([P, QW], f32, tag="y", name="ppq")
                        for c in range(KC):
                            nc.tensor.matmul(
                                pp,
                                wq_ap(c, slice(dc * P, (dc + 1) * P)),
                                xq_t[:, c, tq * QW : (tq + 1) * QW],
                                start=(c == 0),
                                stop=(c == KC - 1),
                            )
                        nc.vector.tensor_scalar_add(
                            qt_sb[:, dc, tq * QW : (tq + 1) * QW],
                            pp,
                            bq_sb[:, dc : dc + 1],
                        )

                    units.append(proj_u)
                return units

            out_po = out_d.rearrange("(k f p) c -> k p f c", p=P, f=4)

            def po_units(tq):
                units = []
                state = {}
                for ts_ in range(4):
                    tch = tq * 4 + ts_
                    for co in range(2):

                        def u(tch=tch, ts_=ts_, co=co):
                            if "o" not in state:
                                state["o"] = outst.tile([P, 4, C], bf16, tag="o", name="o_st")
                            po = ps_y.tile([P, QW], f32, tag="y", name="po")
                            for dc in range(DC):
                                nc.tensor.matmul(
                                    po,
                                    yt_sb[:, dc, tch * P : (tch + 1) * P],
                                    wo_ap(dc, slice(co * QW, (co + 1) * QW)),
                                    start=(dc == 0),
                                    stop=(dc == DC - 1),
                                )
                            nc.vector.tensor_copy(
                                state["o"][:, ts_, co * QW : (co + 1) * QW], po
                            )
                            if ts_ == 3 and co == 1:
                                nc.sync.dma_start(out_po[tq], state["o"])

                        units.append(u)
                return units

            # phase-A prep as a streamable queue: per granule g (512 tok):
            # 4 V-proj chunks + 2 K-proj halves
            prep_q = []
            for g in range(NQ):
                for ts_ in range(4):
                    prep_q.append(vproj_unit(g * 4 + ts_))
                for dc in range(DC):
                    prep_q.append(kproj_unit(g, dc))

            # ---- phase B: attention passes per (tq, head-pair) ----
            y_tiles = {}
            e_tiles = {}

            def emit_sexp(k, hc, tk):
                sp = ps_s.tile([P, 2 * QW], f32, tag="s", name="sp")
                for hh in range(2):
                    nc.tensor.matmul(
                        sp[:, hh * QW : (hh + 1) * QW],
                        kt_sb[hh * 64 : (hh + 1) * 64, hc, tk * P : (tk + 1) * P],
                        qt_sb[hh * 64 : (hh + 1) * 64, hc, k * QW : (k + 1) * QW],
                        start=True,
                        stop=True,
                        tile_position=(hh * 64, 0),
                    )
                e2 = ework.tile([P, 2 * QW], bf16, tag="e", name="e2")
                nc.scalar.activation(e2, sp, Exp, scale=0.125)
                e_tiles[(k, hc, tk)] = e2

            def emit_y(k, hc, tk):
                if (k, hc) not in y_tiles:
                    y_tiles[(k, hc)] = [
                        ps_y.tile([65, QW], f32, tag="y", name=f"y_ps{i}")
                        for i in range(2)
                    ]
                y_pair = y_tiles[(k, hc)]
                e2 = e_tiles.pop((k, hc, tk))
                for hh in range(2):
                    h = 2 * hc + hh
                    nc.tensor.matmul(
                        y_pair[hh],
                        v_sb[:, tk, h, :65],
                        e2[:, hh * QW : (hh + 1) * QW],
                        start=(tk == 0),
                        stop=(tk == NT - 1),
                    )

            def emit_norm(k, hc):
                y_pair = y_tiles.pop((k, hc))
                recr = norm2.tile([P, 2, QW], bf16, tag="recr")
                with nc.allow_low_precision(reason="softmax denom reciprocal"):
                    for hh in range(2):
                        nc.vector.reciprocal(
                            recr[64:65, hh, :], y_pair[hh][64:65, :]
                        )
                rbp = ps_s.tile([P, 2 * QW], f32, tag="s", name="rbp")
                for hh in range(2):
                    nc.tensor.matmul(
                        rbp[0:64, hh * QW : (hh + 1) * QW],
                        onesb[64:65, :],
                        recr[64:65, hh, :],
                        start=True,
                        stop=True,
                        tile_position=(64, 0),
                        skip_group_check=True,
                    )
                rbs = norm2.tile([P, 2 * QW], f32, tag="rbs")
                nc.vector.tensor_copy(rbs[0:64, :], rbp[0:64, :])
                for hh in range(2):
                    rb_h = rbs[0:64, hh * QW : (hh + 1) * QW]
                    if hh == 0:
                        nc.vector.tensor_mul(
                            out=yt_sb[0:64, hc, k * QW : (k + 1) * QW],
                            in0=y_pair[hh][0:64, :],
                            in1=rb_h,
                        )
                    else:
                        yst = norm2.tile([64, QW], bf16, tag="yst")
                        nc.vector.tensor_mul(
                            out=yst, in0=y_pair[hh][0:64, :], in1=rb_h
                        )
                        nc.sync.dma_start(
                            yt_sb[64:128, hc, k * QW : (k + 1) * QW], yst
                        )

            passes = [(k, hc) for k in range(NQ) for hc in range(DC)]
            unit_q = []
            yq = []
            # Q-prep for tq0 first (xq g0 is the first DMA), then granule-0
            # prep so S(0,0,0) has kt/v chunk 0
            for u in q_prep_units(0):
                u()
            for _ in range(6):
                prep_q.pop(0)()

            for pi, (k, hc) in enumerate(passes):
                if hc == 0 and k + 1 < NQ:
                    unit_q.extend(q_prep_units(k + 1))
                for tk in range(NT):
                    emit_sexp(k, hc, tk)
                    yq.append((k, hc, tk))
                    if len(yq) > LAG:
                        emit_y(*yq.pop(0))
                    if tk == 1 and pi >= 1:
                        pk, phc = passes[pi - 1]
                        while yq and yq[0][:2] == (pk, phc):
                            emit_y(*yq.pop(0))
                        emit_norm(pk, phc)
                        if hc == 0 and k >= 1:
                            unit_q.extend(po_units(k - 1))
                    # stream phase-A prep ahead of need during the first pass
                    if prep_q:
                        prep_q.pop(0)()
                        if tk % 2 == 0 and prep_q and pi == 0:
                            prep_q.pop(0)()
                    elif unit_q:
                        unit_q.pop(0)()
            while unit_q:
                unit_q.pop(0)()
            while yq:
                emit_y(*yq.pop(0))
            emit_norm(NQ - 1, DC - 1)
            for u in po_units(NQ - 1):
                u()

            ps_y.release()
            ps_s.release()

    nc.compile()
    return nc


def _get_nc():
    if "nc" not in _CACHE:
        _CACHE["nc"] = _build()
    return _CACHE["nc"]


def _shard_inputs(x_q, x_kv, Wq, bq, Wkv, bkv, Wo):
    import ml_dtypes

    bf16 = ml_dtypes.bfloat16

    def pack_proj(W):  # [C, CL] -> [128, 8*256] in (kc, d) order
        return W.reshape(8, P, CL).transpose(1, 0, 2).reshape(P, 8 * CL)

    in_maps = []
    for core in range(NCORES):
        b = core // TPG
        g = core % TPG
        cols = slice(g * CL, (g + 1) * CL)
        wo_loc = Wo[g * CL : (g + 1) * CL, :]  # [256, 1024]
        wblob1 = pack_proj(Wq[:, cols])
        wblob2 = np.concatenate(
            [
                pack_proj(Wkv[:, :C][:, cols]),
                pack_proj(Wkv[:, C:][:, cols]),
                wo_loc.reshape(2, P, C).transpose(1, 0, 2).reshape(P, 2 * C),
            ],
            axis=1,
        )
        bblob = np.concatenate(
            [
                bq[cols].reshape(2, P).T,
                bkv[:C][cols].reshape(2, P).T,
            ],
            axis=1,
        )
        in_maps.append(
            {
                "xq": np.ascontiguousarray(x_q[b]).astype(bf16),
                "xkv": np.ascontiguousarray(x_kv[b]).astype(bf16),
                "wb1": np.ascontiguousarray(wblob1).astype(bf16),
                "wb2": np.ascontiguousarray(wblob2).astype(bf16),
                "bb": np.ascontiguousarray(bblob).astype(np.float32),
            }
        )
    return in_maps


def kernel(x_q, x_kv, Wq, bq, Wkv, bkv, Wo, bo):
    from concourse.bass_utils import run_bass_kernel_spmd

    x_q = np.asarray(x_q, dtype=np.float32)
    x_kv = np.asarray(x_kv, dtype=np.float32)
    Wq = np.asarray(Wq, dtype=np.float32)
    bq = np.asarray(bq, dtype=np.float32)
    Wkv = np.asarray(Wkv, dtype=np.float32)
    bkv = np.asarray(bkv, dtype=np.float32)
    Wo = np.asarray(Wo, dtype=np.float32)
    bo = np.asarray(bo, dtype=np.float32)

    nc = _get_nc()
    in_maps = _shard_inputs(x_q, x_kv, Wq, bq, Wkv, bkv, Wo)
    res = run_bass_kernel_spmd(nc, in_maps, core_ids=list(range(NCORES)))

    # host-side gather: sum tensor-parallel partials; add exact bias terms
    bias_full = bkv[C:] @ Wo + bo  # v-bias through Wo, plus output bias
    out = np.zeros((B, T, C), dtype=np.float32)
    for core in range(NCORES):
        out[core // TPG] += np.asarray(res.results[core]["out"]).astype(np.float32)
    out += bias_full[None, None, :]
    return out


# revision 26
# speedup vs baseline: 1.7715x; 1.0159x over previous
"""Cross-attention Bass/Tile kernel for Trainium2, sharded over 8 NeuronCores.

Problem (fixed shapes): B=2, T=2048, C=1024, H=16 heads, D=64.
    q = x_q @ Wq + bq;  kv = x_kv @ Wkv + bkv;  k, v = split(kv)
    y = softmax(q k^T / sqrt(D)) v;  out = y @ Wo + bo

Sharding: 8 cores = 2 (batch) x 4 (head groups of 4 heads, 256 channels).
Each core computes its head-group's projections + attention + a partial
output projection (its 256 rows of Wo); the host sums the 4 partials per
batch.  The v-bias and output bias are folded in exactly on the host:
    y = att@(V + 1*bv) = att@V + 1*bv   (att rows sum to 1)
    => out += bv @ Wo + bo              (added once per batch on the host)

v3 (over the f32r baseline):
  - bf16 operands everywhere (x and weights staged bf16 from host):
    halves DMA traffic; psum stays f32.
  - x^T via DMA-transpose (XBAR, 16x128 tiles) straight from DRAM to
    SBUF: eliminates all PE transposes (~49k cycles) and the DVE
    psum->SBUF copy-outs (~34us).
  - K projection at 512-token granularity (64 instead of 128 matmuls).
  - Output partials stored bf16 (halves store DMA).
  - Phase A (K/V prep) streams into the first attention pass as woven
    units instead of a serial prologue.

Attention per (tq 512-block, head-pair) pass, per tk chunk: S^T matmul
(2 heads row-packed via tile_position) -> exp on ACT (scale=1/8) ->
att@V matmuls lagging LAG units.  V carries a ones column so row 64 of
the y psum accumulates the softmax denominator; normalization is
reciprocal + K=1 broadcast matmul + DVE multiply (baseline-proven).
PE matmul count kept low (~850): the PE sequencer costs ~130ns per
instruction (SW decode), which is the binding constraint before engine
cycles for narrow matmuls.

PSUM (8 banks): 2 x [128,1024] "s" + 4 x [128,512] "y" slots shared by
y-accumulators and woven work units (baseline-proven rotation).
"""

import numpy as np

B = 2
T = 2048
C = 1024
H = 16
D = 64
NCORES = 8
TPG = 4  # tensor-parallel group size (head groups)
HL = H // TPG  # heads per core = 4
CL = HL * D  # local channels = 256
P = 128

_CACHE = {}


def _build(debug=False):
    import concourse.tile as tile
    from concourse import bacc, mybir

    f32 = mybir.dt.float32
    bf16 = mybir.dt.bfloat16
    Exp = mybir.ActivationFunctionType.Exp

    nc = bacc.Bacc("TRN2", target_bir_lowering=False, debug=False)

    xq_d = nc.dram_tensor("xq", [T, C], bf16, kind="ExternalInput")
    xkv_d = nc.dram_tensor("xkv", [T, C], bf16, kind="ExternalInput")
    # weights prepacked on host into two bf16 blobs: wb1=[wq 8x256],
    # wb2=[wk 8x256 | wv 8x256 | wo 2x1024] per partition row (wb1 first
    # so Q-prep's DMA chain is short)
    wb1_d = nc.dram_tensor("wb1", [P, 2048], bf16, kind="ExternalInput")
    wb2_d = nc.dram_tensor("wb2", [P, 6144], bf16, kind="ExternalInput")
    bb_d = nc.dram_tensor("bb", [P, 4], f32, kind="ExternalInput")
    out_d = nc.dram_tensor("out", [T, C], bf16, kind="ExternalOutput")

    KC = C // P  # 8 contraction chunks for the projections
    NT = T // P  # 16 token chunks of 128
    NQ = 4  # tq blocks of 512
    QW = T // NQ  # 512
    DC = CL // P  # 2 chunks of d_local
    LAG = 4

    with tile.TileContext(nc) as tc:
        with (
            tc.tile_pool(name="const", bufs=1) as const,
            tc.tile_pool(name="persist", bufs=1) as persist,
            tc.tile_pool(name="ework", bufs=7) as ework,
            tc.tile_pool(name="norm2", bufs=1) as norm2,
            tc.tile_pool(name="outst", bufs=3) as outst,
        ):
            ones4 = const.tile([P, HL, 1], bf16)
            nc.vector.memset(ones4, 1.0)
            onesb = const.tile([P, 64], bf16)
            nc.vector.memset(onesb, 1.0)

            # ---- weights: ONE blob DMA + one bias DMA (DMA instructions
            # issue serially at ~2.7us each; count is precious) ----
            bb_sb = const.tile([P, 4], f32)
            wb1_sb = const.tile([P, 2048], bf16)
            nc.gpsimd.dma_start(wb1_sb, wb1_d[:, :])
            wb2_sb = const.tile([P, 6144], bf16)
            bq_sb = bb_sb[:, 0:2]
            bk_sb = bb_sb[:, 2:4]

            def wq_ap(kc, sl):
                return wb1_sb[:, kc * CL + sl.start : kc * CL + sl.stop]

            def wk_ap(kc, sl):
                return wb2_sb[:, kc * CL + sl.start : kc * CL + sl.stop]

            def wv_ap(kc):
                return wb2_sb[:, 2048 + kc * CL : 2048 + (kc + 1) * CL]

            def wo_ap(dc, sl):
                return wb2_sb[:, 4096 + dc * C + sl.start : 4096 + dc * C + sl.stop]

            # ---- persistent activations ----
            xq_t = persist.tile([P, KC, T], bf16)  # xq^T  [c, t]
            xkv_t = persist.tile([P, KC, T], bf16)  # xkv^T [c, t]
            qt_sb = persist.tile([P, DC, T], bf16)  # Q^T  [d, t]
            kt_sb = persist.tile([P, DC, T], bf16)  # K^T  [d, t]
            v_sb = persist.tile([P, NT, HL, 66], bf16)  # V|1 [t, h, d+1]
            yt_sb = persist.tile([P, DC, T], bf16)  # y^T  [d, t] (normalized)

            # ---- input transposes (XBAR DMA): ONE [512,1024] DMA per
            # granule covers all 8 c-chunks -> out[p, c, t] = x^T[c*128+p, t]
            def emit_xT(dst, src_d, g):
                t0 = g * QW
                nc.sync.dma_start(
                    dst[:, :, t0 : t0 + QW],
                    src_d[t0 : t0 + QW, :],
                    transpose=True,
                )

            emit_xT(xq_t, xq_d, 0)
            nc.gpsimd.dma_start(bb_sb, bb_d[:, :])
            nc.gpsimd.dma_start(wb2_sb, wb2_d[:, :])
            emit_xT(xkv_t, xkv_d, 0)
            for g in range(1, NQ):
                emit_xT(xkv_t, xkv_d, g)
            for g in range(1, NQ):
                emit_xT(xq_t, xq_d, g)

            # ---- kernel-wide PSUM: 2 x [128,1024] (s) + 4 x [128,512] (y)
            ps_s = tc.alloc_tile_pool(name="ps_s", bufs=2, space="PSUM")
            ps_y = tc.alloc_tile_pool(name="ps_y", bufs=4, space="PSUM")

            # ---------- emission helpers ----------
            def vproj_unit(tch):
                def u():
                    pv = ps_y.tile([P, QW], f32, tag="y", name="pv")
                    for c in range(KC):
                        nc.tensor.matmul(
                            pv[:, :CL],
                            xkv_t[:, c, tch * P : (tch + 1) * P],
                            wv_ap(c),
                            start=(c == 0),
                            stop=(c == KC - 1),
                        )
                    nc.vector.tensor_copy(
                        v_sb[:, tch, :, 0:64],
                        pv[:, :CL].rearrange("p (h d) -> p h d", h=HL),
                    )
                    nc.vector.tensor_copy(v_sb[:, tch, :, 64:65], ones4)

                return u

            def kproj_unit(g, dc):
                def u():
                    pp = ps_y.tile([P, QW], f32, tag="y", name="ppk")
                    for c in range(KC):
                        nc.tensor.matmul(
                            pp,
                            wk_ap(c, slice(dc * P, (dc + 1) * P)),
                            xkv_t[:, c, g * QW : (g + 1) * QW],
                            start=(c == 0),
                            stop=(c == KC - 1),
                        )
                    nc.vector.tensor_scalar_add(
                        kt_sb[:, dc, g * QW : (g + 1) * QW],
                        pp,
                        bk_sb[:, dc : dc + 1],
                    )

                return u

            def q_prep_units(tq):
                units = []
                for dc in range(DC):

                    def proj_u(dc=dc):
                        pp = ps_y.tile([P, QW], f32, tag="y", name="ppq")
                        for c in range(KC):
                            nc.tensor.matmul(
                                pp,
                                wq_ap(c, slice(dc * P, (dc + 1) * P)),
                                xq_t[:, c, tq * QW : (tq + 1) * QW],
                                start=(c == 0),
                                stop=(c == KC - 1),
                            )
                        nc.vector.tensor_scalar_add(
                            qt_sb[:, dc, tq * QW : (tq + 1) * QW],
                            pp,
                            bq_sb[:, dc : dc + 1],
                        )

                    units.append(proj_u)
                return units

            out_po = out_d.rearrange("(k f p) c -> k p f c", p=P, f=4)
            out_pq = out_d.rearrange("(t p) c -> t p c", p=P)

            def po_units(tq):
                units = []
                state = {}
                for ts_ in range(4):
                    tch = tq * 4 + ts_
                    for co in range(2):

                        def u(tch=tch, ts_=ts_, co=co):
                            if "o" not in state:
                                state["o"] = outst.tile([P, 4, C], bf16, tag="o", name="o_st")
                            po = ps_y.tile([P, QW], f32, tag="y", name="po")
                            for dc in range(DC):
                                nc.tensor.matmul(
                                    po,
                                    yt_sb[:, dc, tch * P : (tch + 1) * P],
                                    wo_ap(dc, slice(co * QW, (co + 1) * QW)),
                                    start=(dc == 0),
                                    stop=(dc == DC - 1),
                                )
                            nc.vector.tensor_copy(
                                state["o"][:, ts_, co * QW : (co + 1) * QW], po
                            )
                            if tq == NQ - 1:
                                if co == 1:
                                    nc.sync.dma_start(
                                        out_pq[tch], state["o"][:, ts_, :]
                                    )
                            elif ts_ == 3 and co == 1:
                                nc.sync.dma_start(out_po[tq], state["o"])

                        units.append(u)
                return units

            # phase-A prep as a streamable queue: per granule g (512 tok):
            # 4 V-proj chunks + 2 K-proj halves
            prep_q = []
            for g in range(NQ):
                for ts_ in range(4):
                    prep_q.append(vproj_unit(g * 4 + ts_))
                for dc in range(DC):
                    prep_q.append(kproj_unit(g, dc))

            # ---- phase B: attention passes per (tq, head-pair) ----
            y_tiles = {}
            e_tiles = {}

            def emit_sexp(k, hc, tk):
                sp = ps_s.tile([P, 2 * QW], f32, tag="s", name="sp")
                for hh in range(2):
                    nc.tensor.matmul(
                        sp[:, hh * QW : (hh + 1) * QW],
                        kt_sb[hh * 64 : (hh + 1) * 64, hc, tk * P : (tk + 1) * P],
                        qt_sb[hh * 64 : (hh + 1) * 64, hc, k * QW : (k + 1) * QW],
                        start=True,
                        stop=True,
                        tile_position=(hh * 64, 0),
                    )
                e2 = ework.tile([P, 2 * QW], bf16, tag="e", name="e2")
                nc.scalar.activation(e2, sp, Exp, scale=0.125)
                e_tiles[(k, hc, tk)] = e2

            def emit_y(k, hc, tk):
                if (k, hc) not in y_tiles:
                    y_tiles[(k, hc)] = [
                        ps_y.tile([65, QW], f32, tag="y", name=f"y_ps{i}")
                        for i in range(2)
                    ]
                y_pair = y_tiles[(k, hc)]
                e2 = e_tiles.pop((k, hc, tk))
                for hh in range(2):
                    h = 2 * hc + hh
                    nc.tensor.matmul(
                        y_pair[hh],
                        v_sb[:, tk, h, :65],
                        e2[:, hh * QW : (hh + 1) * QW],
                        start=(tk == 0),
                        stop=(tk == NT - 1),
                    )

            def emit_norm(k, hc):
                y_pair = y_tiles.pop((k, hc))
                recr = norm2.tile([P, 2, QW], bf16, tag="recr")
                with nc.allow_low_precision(reason="softmax denom reciprocal"):
                    for hh in range(2):
                        nc.vector.reciprocal(
                            recr[64:65, hh, :], y_pair[hh][64:65, :]
                        )
                rbp = ps_s.tile([P, 2 * QW], f32, tag="s", name="rbp")
                for hh in range(2):
                    nc.tensor.matmul(
                        rbp[0:64, hh * QW : (hh + 1) * QW],
                        onesb[64:65, :],
                        recr[64:65, hh, :],
                        start=True,
                        stop=True,
                        tile_position=(64, 0),
                        skip_group_check=True,
                    )
                rbs = norm2.tile([P, 2 * QW], f32, tag="rbs")
                nc.vector.tensor_copy(rbs[0:64, :], rbp[0:64, :])
                for hh in range(2):
                    rb_h = rbs[0:64, hh * QW : (hh + 1) * QW]
                    if hh == 0:
                        nc.vector.tensor_mul(
                            out=yt_sb[0:64, hc, k * QW : (k + 1) * QW],
                            in0=y_pair[hh][0:64, :],
                            in1=rb_h,
                        )
                    else:
                        yst = norm2.tile([64, QW], bf16, tag="yst")
                        nc.vector.tensor_mul(
                            out=yst, in0=y_pair[hh][0:64, :], in1=rb_h
                        )
                        nc.sync.dma_start(
                            yt_sb[64:128, hc, k * QW : (k + 1) * QW], yst
                        )

            passes = [(k, hc) for k in range(NQ) for hc in range(DC)]
            unit_q = []
            yq = []
            # Q-prep for tq0 first (xq g0 is the first DMA), then granule-0
            # prep so S(0,0,0) has kt/v chunk 0
            for u in q_prep_units(0):
                u()
            for _ in range(6):
                prep_q.pop(0)()

            for pi, (k, hc) in enumerate(passes):
                if hc == 0 and k + 1 < NQ:
                    unit_q.extend(q_prep_units(k + 1))
                for tk in range(NT):
                    emit_sexp(k, hc, tk)
                    yq.append((k, hc, tk))
                    if len(yq) > LAG:
                        emit_y(*yq.pop(0))
                    if tk == 1 and pi >= 1:
                        pk, phc = passes[pi - 1]
                        while yq and yq[0][:2] == (pk, phc):
                            emit_y(*yq.pop(0))
                        emit_norm(pk, phc)
                        if hc == 0 and k >= 1:
                            unit_q.extend(po_units(k - 1))
                    # stream phase-A prep ahead of need during the first pass
                    if prep_q:
                        prep_q.pop(0)()
                        if tk % 2 == 0 and prep_q and pi == 0:
                            prep_q.pop(0)()
                    elif unit_q:
                        unit_q.pop(0)()
            while unit_q:
                unit_q.pop(0)()
            while yq:
                emit_y(*yq.pop(0))
            emit_norm(NQ - 1, DC - 1)
            for u in po_units(NQ - 1):
                u()

            ps_y.release()
            ps_s.release()

    nc.compile()
    return nc


def _get_nc():
    if "nc" not in _CACHE:
        _CACHE["nc"] = _build()
    return _CACHE["nc"]


def _shard_inputs(x_q, x_kv, Wq, bq, Wkv, bkv, Wo):
    import ml_dtypes

    bf16 = ml_dtypes.bfloat16

    def pack_proj(W):  # [C, CL] -> [128, 8*256] in (kc, d) order
        return W.reshape(8, P, CL).transpose(1, 0, 2).reshape(P, 8 * CL)

    in_maps = []
    for core in range(NCORES):
        b = core // TPG
        g = core % TPG
        cols = slice(g * CL, (g + 1) * CL)
        wo_loc = Wo[g * CL : (g + 1) * CL, :]  # [256, 1024]
        wblob1 = pack_proj(Wq[:, cols])
        wblob2 = np.concatenate(
            [
                pack_proj(Wkv[:, :C][:, cols]),
                pack_proj(Wkv[:, C:][:, cols]),
                wo_loc.reshape(2, P, C).transpose(1, 0, 2).reshape(P, 2 * C),
            ],
            axis=1,
        )
        bblob = np.concatenate(
            [
                bq[cols].reshape(2, P).T,
                bkv[:C][cols].reshape(2, P).T,
            ],
            axis=1,
        )
        in_maps.append(
            {
                "xq": np.ascontiguousarray(x_q[b]).astype(bf16),
                "xkv": np.ascontiguousarray(x_kv[b]).astype(bf16),
                "wb1": np.ascontiguousarray(wblob1).astype(bf16),
                "wb2": np.ascontiguousarray(wblob2).astype(bf16),
                "bb": np.ascontiguousarray(bblob).astype(np.float32),
            }
        )
    return in_maps


def kernel(x_q, x_kv, Wq, bq, Wkv, bkv, Wo, bo):
    from concourse.bass_utils import run_bass_kernel_spmd

    x_q = np.asarray(x_q, dtype=np.float32)
    x_kv = np.asarray(x_kv, dtype=np.float32)
    Wq = np.asarray(Wq, dtype=np.float32)
    bq = np.asarray(bq, dtype=np.float32)
    Wkv = np.asarray(Wkv, dtype=np.float32)
    bkv = np.asarray(bkv, dtype=np.float32)
    Wo = np.asarray(Wo, dtype=np.float32)
    bo = np.asarray(bo, dtype=np.float32)

    nc = _get_nc()
    in_maps = _shard_inputs(x_q, x_kv, Wq, bq, Wkv, bkv, Wo)
    res = run_bass_kernel_spmd(nc, in_maps, core_ids=list(range(NCORES)))

    # host-side gather: sum tensor-parallel partials; add exact bias terms
    bias_full = bkv[C:] @ Wo + bo  # v-bias through Wo, plus output bias
    out = np.zeros((B, T, C), dtype=np.float32)
    for core in range(NCORES):
        out[core // TPG] += np.asarray(res.results[core]["out"]).astype(np.float32)
    out += bias_full[None, None, :]
    return out
